# revision 8
# baseline (speedup 1.0000x reference)
"""Self-contained Trainium2 kernel for the BotImpact GNN (2x GATConv on two
graphs + MLP heads), SPMD across 8 NeuronCores.

Strategy (1D node sharding):
 - core k owns nodes [k*6272, (k+1)*6272) of the zero-padded node space
   (N padded 50000 -> 50176 so every core owns exactly 49 windows of 128).
 - Per conv: dense phase computes H rows [h(64) | 1.0 | a_s | a_d | pad] for
   ALL nodes on every core (replicated compute, inputs host-transposed);
   edge phase gathers H[src] rows per edge with dma_gather (int16 indices ->
   table split in two halves at row 25088, each with a trailing zero pad row)
   and aggregates the segment softmax per 128-dst-node window with selection
   -matrix matmuls on the TensorEngine accumulating in PSUM:
       out[slot,:] = sum_e M01[e,slot] * (exp(lrelu(a_s[src]+a_d[dst])) * [h|1])
   then out = num/den (softmax denominator = the appended ones column).
 - Halo exchange between convs is done on the host (3 launches):
   A: conv1 (both graphs) -> B: conv2 + tprob -> C: heads (host pre-gathers
   treat/control rows; device runs the MLPs).
"""

import os
import numpy as np

import concourse.bacc as bacc
import concourse.bass as bass
import concourse.mybir as mybir
import concourse.tile as tile
from concourse.bass_utils import run_bass_kernel_spmd
from concourse.masks import make_identity

F32 = mybir.dt.float32
I16 = mybir.dt.int16

NCORES = 8
N_REAL = 50000
NPAD = 50176            # 392 tiles of 128
NOWN = NPAD // NCORES   # 6272
NW = NOWN // 128        # 49 windows per core
NTILE = NPAD // 128     # 392
HALF = NPAD // 2        # 25088  (= 49*512, quad aligned)
ROW = 128               # H row width (f32) = 512B
FD = 64                 # feature dim
IN_DIM = 128
T_CNT = 25000
HEAD_PAD = ((T_CNT // NCORES) + 127) // 128 * 128  # 3200 per core
SLOPE_GAT = 0.2
SLOPE_MLP = 0.01
GCHUNK_TILES = 8        # <=1024 idx per dma_gather

# H table rows: [0,25088) lo | 25088 zero | [25089,50177) hi | 50177 zero
HROWS = NPAD + 2
LO_DUMMY = HALF         # idx 25088 -> zero row of lo table
HI_DUMMY = HALF         # idx 25088 -> zero row of hi table (row 50177)

LAST_EXEC_NS = []       # per-launch HW exec times (filled when tracing)
_TRACE = bool(int(os.environ.get("KERNEL_TRACE", "0")))


# ----------------------------------------------------------------- host prep

def _pack_idx(idx_tiles):
    """[T,128] int array -> [128, T*8] int16 wrapped mod 16, replicated x8."""
    t = idx_tiles.shape[0]
    out = np.zeros((128, t * 8), dtype=np.int16)
    for ti in range(t):
        w = idx_tiles[ti].reshape(8, 16).T.astype(np.int16)  # [16, 8]
        out[:, ti * 8:(ti + 1) * 8] = np.tile(w, (8, 1))
    return out


def prep_graph(edge_index):
    """Returns (meta, per_core_arrays).

    meta: dict with 'ct' [NW,2] common tile counts per (window, half).
    per core: idx_lo [128,TLlo*8] i16, slot_lo [128,TLlo] f32, idx_hi,
    slot_hi, own_lo [128,NW*8] i16, own_hi [128,NW*8] i16.
    """
    ei = np.asarray(edge_index).astype(np.int64)
    loop = np.arange(N_REAL, dtype=np.int64)
    src = np.concatenate([ei[0], loop])
    dst = np.concatenate([ei[1], loop])

    per_core_edges = []
    counts = np.zeros((NCORES, NW, 2), dtype=np.int64)
    for k in range(NCORES):
        m = (dst // NOWN) == k
        s, d = src[m], dst[m] - k * NOWN
        w = d >> 7
        slot = d & 127
        half = (s >= HALF).astype(np.int64)
        order = np.lexsort((half, w))
        s, w, slot, half = s[order], w[order], slot[order], half[order]
        per_core_edges.append((s, w, slot, half))
        for wi in range(NW):
            wm = w == wi
            counts[k, wi, 0] = np.count_nonzero(wm & (half == 0))
            counts[k, wi, 1] = np.count_nonzero(wm & (half == 1))

    ct = np.zeros((NW, 2), dtype=np.int64)
    for wi in range(NW):
        for h in range(2):
            ct[wi, h] = -(-counts[:, wi, h].max() // 128)  # ceil

    tl_lo, tl_hi = int(ct[:, 0].sum()), int(ct[:, 1].sum())
    cores = []
    for k in range(NCORES):
        s, w, slot, half = per_core_edges[k]
        idx_lo = np.full((tl_lo * 128,), LO_DUMMY, dtype=np.int64)
        slot_lo = np.full((tl_lo * 128,), -1.0, dtype=np.float32)
        idx_hi = np.full((tl_hi * 128,), HI_DUMMY, dtype=np.int64)
        slot_hi = np.full((tl_hi * 128,), -1.0, dtype=np.float32)
        off = {0: 0, 1: 0}
        for wi in range(NW):
            for h, (idx_a, slot_a) in ((0, (idx_lo, slot_lo)),
                                       (1, (idx_hi, slot_hi))):
                m = (w == wi) & (half == h)
                se = s[m] - (HALF if h else 0)
                sl = slot[m].astype(np.float32)
                n = se.shape[0]
                idx_a[off[h]:off[h] + n] = se
                slot_a[off[h]:off[h] + n] = sl
                off[h] += int(ct[wi, h]) * 128
        # own-node gathers: window wi -> global ids k*NOWN + 128*wi + p
        own_lo = np.full((NW, 128), LO_DUMMY, dtype=np.int64)
        own_hi = np.full((NW, 128), HI_DUMMY, dtype=np.int64)
        for wi in range(NW):
            ids = k * NOWN + 128 * wi + np.arange(128)
            lo_m = ids < HALF
            own_lo[wi, lo_m] = ids[lo_m]
            own_hi[wi, ~lo_m] = ids[~lo_m] - HALF
        cores.append({
            "idx_lo": _pack_idx(idx_lo.reshape(tl_lo, 128)),
            "slot_lo": slot_lo.reshape(tl_lo, 128).T.copy(),  # [128, TLlo]
            "idx_hi": _pack_idx(idx_hi.reshape(tl_hi, 128)),
            "slot_hi": slot_hi.reshape(tl_hi, 128).T.copy(),
            "own_lo": _pack_idx(own_lo),
            "own_hi": _pack_idx(own_hi),
        })
    meta = {"ct": ct, "tl_lo": tl_lo, "tl_hi": tl_hi}
    return meta, cores


# ------------------------------------------------------------- conv builder

def _chunks(n_tiles):
    out = []
    t = 0
    while t < n_tiles:
        c = min(GCHUNK_TILES, n_tiles - t)
        out.append((t, c))
        t += c
    return out


def build_conv_launch(kdim, metas, relu_out, with_tprob):
    """One conv layer applied to both graphs. metas = [meta_r, meta_f].

    Inputs (per core): xT_r [kdim,NPAD], xT_f, W [kdim,64], asr/adr/br
    [128,64] (replicated rows), iota [128,128], per-graph edge streams.
    Outputs: xz_r / xz_f [128, NW*64]  (+ tprobT [2, NOWN] if with_tprob).
    """
    nc = bacc.Bacc("TRN2", target_bir_lowering=False)
    g_names = ["r", "f"]
    xT = {g: nc.dram_tensor(f"xT_{g}", [kdim, NPAD], F32, kind="ExternalInput")
          for g in g_names}
    W_in = nc.dram_tensor("W", [kdim, FD], F32, kind="ExternalInput")
    as_in = nc.dram_tensor("as_r", [128, FD], F32, kind="ExternalInput")
    ad_in = nc.dram_tensor("ad_r", [128, FD], F32, kind="ExternalInput")
    b_in = nc.dram_tensor("b_r", [128, FD], F32, kind="ExternalInput")
    iota_in = nc.dram_tensor("iota", [128, 128], F32, kind="ExternalInput")
    if with_tprob:
        wp_in = nc.dram_tensor("Wp", [FD, 2], F32, kind="ExternalInput")
        bp_in = nc.dram_tensor("bp", [128, 1], F32, kind="ExternalInput")

    streams = {}
    for g, meta in zip(g_names, metas):
        tl_lo, tl_hi = meta["tl_lo"], meta["tl_hi"]
        streams[g] = {
            "idx_lo": nc.dram_tensor(f"idx_lo_{g}", [128, tl_lo * 8], I16,
                                     kind="ExternalInput"),
            "slot_lo": nc.dram_tensor(f"slot_lo_{g}", [128, tl_lo], F32,
                                      kind="ExternalInput"),
            "idx_hi": nc.dram_tensor(f"idx_hi_{g}", [128, tl_hi * 8], I16,
                                     kind="ExternalInput"),
            "slot_hi": nc.dram_tensor(f"slot_hi_{g}", [128, tl_hi], F32,
                                      kind="ExternalInput"),
            "own_lo": nc.dram_tensor(f"own_lo_{g}", [128, NW * 8], I16,
                                     kind="ExternalInput"),
            "own_hi": nc.dram_tensor(f"own_hi_{g}", [128, NW * 8], I16,
                                     kind="ExternalInput"),
        }

    H = {g: nc.dram_tensor(f"H_{g}", [HROWS, ROW], F32) for g in g_names}
    xz_out = {g: nc.dram_tensor(f"xz_{g}", [128, NW * FD], F32,
                                kind="ExternalOutput") for g in g_names}
    if with_tprob:
        tp_out = nc.dram_tensor("tprobT", [2, NOWN], F32, kind="ExternalOutput")

    from contextlib import ExitStack
    with tile.TileContext(nc) as tc, ExitStack() as es:
        cpool = es.enter_context(tc.tile_pool(name="const", bufs=1))
        spool = es.enter_context(tc.tile_pool(name="streams", bufs=1))
        dq = es.enter_context(tc.tile_pool(name="densequad", bufs=3))
        dps = es.enter_context(tc.tile_pool(name="densepsum", bufs=2, space="PSUM"))
        mps = es.enter_context(tc.tile_pool(name="miscpsum", bufs=1, space="PSUM"))
        gp = es.enter_context(tc.tile_pool(name="gtiles", bufs=4))
        mp = es.enter_context(tc.tile_pool(name="m01", bufs=4))
        tp = es.enter_context(tc.tile_pool(name="tmp", bufs=4))
        wp_ = es.enter_context(tc.tile_pool(name="wpsum", bufs=2, space="PSUM"))
        pp = es.enter_context(tc.tile_pool(name="post", bufs=4))
        xp = es.enter_context(tc.tile_pool(name="xzown", bufs=1))

        W_sb = cpool.tile([kdim, FD], F32)
        nc.sync.dma_start(out=W_sb[:], in_=W_in[:])
        as_sb = cpool.tile([128, FD], F32)
        nc.sync.dma_start(out=as_sb[:], in_=as_in[:])
        ad_sb = cpool.tile([128, FD], F32)
        nc.sync.dma_start(out=ad_sb[:], in_=ad_in[:])
        b_sb = cpool.tile([128, FD], F32)
        nc.sync.dma_start(out=b_sb[:], in_=b_in[:])
        iota_sb = cpool.tile([128, 128], F32)
        nc.sync.dma_start(out=iota_sb[:], in_=iota_in[:])
        ones_row = cpool.tile([1, 128], F32)
        nc.vector.memset(ones_row[:], 1.0)
        ident = cpool.tile([128, 128], F32)
        make_identity(nc, ident[:])
        if with_tprob:
            wp_sb = cpool.tile([FD, 2], F32)
            nc.sync.dma_start(out=wp_sb[:], in_=wp_in[:])
            bp_sb = cpool.tile([128, 1], F32)
            nc.sync.dma_start(out=bp_sb[:], in_=bp_in[:])

        # zero the two pad rows of each H table
        for g in g_names:
            zrow = cpool.tile([1, ROW], F32, tag="zrow")
            nc.vector.memset(zrow[:], 0.0)
            nc.sync.dma_start(out=H[g][HALF:HALF + 1, :], in_=zrow[:])
            nc.sync.dma_start(out=H[g][HROWS - 1:HROWS, :], in_=zrow[:])

        for g, meta in zip(g_names, metas):
            st = streams[g]
            tl_lo, tl_hi = meta["tl_lo"], meta["tl_hi"]
            ct = meta["ct"]

            idx_lo_sb = spool.tile([128, tl_lo * 8], I16, tag=f"il{g}")
            nc.sync.dma_start(out=idx_lo_sb[:], in_=st["idx_lo"][:])
            slot_lo_sb = spool.tile([128, tl_lo], F32, tag=f"sl{g}")
            nc.sync.dma_start(out=slot_lo_sb[:], in_=st["slot_lo"][:])
            idx_hi_sb = spool.tile([128, tl_hi * 8], I16, tag=f"ih{g}")
            nc.sync.dma_start(out=idx_hi_sb[:], in_=st["idx_hi"][:])
            slot_hi_sb = spool.tile([128, tl_hi], F32, tag=f"sh{g}")
            nc.sync.dma_start(out=slot_hi_sb[:], in_=st["slot_hi"][:])
            own_lo_sb = spool.tile([128, NW * 8], I16, tag=f"ol{g}")
            nc.sync.dma_start(out=own_lo_sb[:], in_=st["own_lo"][:])
            own_hi_sb = spool.tile([128, NW * 8], I16, tag=f"oh{g}")
            nc.sync.dma_start(out=own_hi_sb[:], in_=st["own_hi"][:])

            # ---------------- dense: H rows for all nodes (quads of 4 tiles)
            for q in range(NTILE // 4):
                lt = dq.tile([kdim, 512], F32, tag="lhs")
                nc.sync.dma_start(out=lt[:], in_=xT[g][:, q * 512:(q + 1) * 512])
                hp = dps.tile([128, 4, FD], F32, space="PSUM", tag="hps")
                for j in range(4):
                    nc.tensor.matmul(
                        out=hp[:, j, :], lhsT=lt[:, j * 128:(j + 1) * 128],
                        rhs=W_sb[:], start=True, stop=True)
                hq = dq.tile([128, 4, ROW], F32, tag="hq")
                nc.vector.memset(hq[:], 0.0)
                nc.vector.tensor_copy(out=hq[:, :, 0:FD], in_=hp[:])
                nc.vector.memset(hq[:, :, FD:FD + 1], 1.0)
                tmp = dq.tile([128, 4, FD], F32, tag="dtmp")
                nc.vector.tensor_tensor(
                    out=tmp[:], in0=hq[:, :, 0:FD],
                    in1=as_sb[:, None, :].to_broadcast([128, 4, FD]),
                    op=mybir.AluOpType.mult)
                nc.vector.tensor_reduce(
                    out=hq[:, :, FD + 1:FD + 2], in_=tmp[:],
                    axis=mybir.AxisListType.X, op=mybir.AluOpType.add)
                nc.vector.tensor_tensor(
                    out=tmp[:], in0=hq[:, :, 0:FD],
                    in1=ad_sb[:, None, :].to_broadcast([128, 4, FD]),
                    op=mybir.AluOpType.mult)
                nc.vector.tensor_reduce(
                    out=hq[:, :, FD + 2:FD + 3], in_=tmp[:],
                    axis=mybir.AxisListType.X, op=mybir.AluOpType.add)
                row0 = q * 512 + (1 if q * 512 >= HALF else 0)
                nc.sync.dma_start(
                    out=H[g][row0:row0 + 512, :].rearrange(
                        "(t p) r -> p t r", p=128),
                    in_=hq[:])

            h_lo = H[g][0:HALF + 1, :]
            h_hi = H[g][HALF + 1:HROWS, :]

            # ---------------- edge phase
            xz_sb = xp.tile([128, NW * FD], F32, tag=f"xz{g}")
            lo_off = np.concatenate([[0], np.cumsum(ct[:, 0])]).astype(int)
            hi_off = np.concatenate([[0], np.cumsum(ct[:, 1])]).astype(int)
            for w in range(NW):
                # own-node rows -> a_d column for this window
                go_l = gp.tile([128, 1, ROW], F32, tag="gown")
                nc.gpsimd.dma_gather(
                    go_l[:], h_lo[:], own_lo_sb[:, w * 8:(w + 1) * 8],
                    128, 128, ROW)
                go_h = gp.tile([128, 1, ROW], F32, tag="gown")
                nc.gpsimd.dma_gather(
                    go_h[:], h_hi[:], own_hi_sb[:, w * 8:(w + 1) * 8],
                    128, 128, ROW)
                adcol = pp.tile([128, 1], F32, tag="adcol")
                nc.vector.tensor_tensor(
                    out=adcol[:], in0=go_l[:, 0, FD + 2:FD + 3],
                    in1=go_h[:, 0, FD + 2:FD + 3], op=mybir.AluOpType.add)
                # transpose [128,1] -> [1,128], then K=1 matmul to replicate
                adT = mps.tile([1, 128], F32, space="PSUM", tag="adT")
                nc.tensor.transpose(out=adT[:], in_=adcol[:], identity=ident[:])
                adT_sb = pp.tile([1, 128], F32, tag="adTs")
                nc.vector.tensor_copy(out=adT_sb[:], in_=adT[:])
                adrep_ps = mps.tile([128, 128], F32, space="PSUM", tag="adrep")
                nc.tensor.matmul(out=adrep_ps[:], lhsT=ones_row[:],
                                 rhs=adT_sb[:], start=True, stop=True)
                adrep = pp.tile([128, 128], F32, tag="adrep_sb")
                nc.vector.tensor_copy(out=adrep[:], in_=adrep_ps[:])

                psw = wp_.tile([128, FD + 1], F32, space="PSUM", tag="psw")
                n_mm = int(ct[w, 0] + ct[w, 1])
                mm_i = 0
                for h_ix, (idx_sb, slot_sb, tbl, off_a) in enumerate(
                        ((idx_lo_sb, slot_lo_sb, h_lo, lo_off),
                         (idx_hi_sb, slot_hi_sb, h_hi, hi_off))):
                    for (t0, cT) in _chunks(int(ct[w, h_ix])):
                        base = int(off_a[w]) + t0
                        g_t = gp.tile([128, GCHUNK_TILES, ROW], F32, tag="g")
                        nc.gpsimd.dma_gather(
                            g_t[:, 0:cT, :], tbl[:],
                            idx_sb[:, base * 8:(base + cT) * 8],
                            cT * 128, cT * 128, ROW)
                        m01 = mp.tile([128, GCHUNK_TILES, 128], F32, tag="m")
                        nc.vector.tensor_tensor(
                            out=m01[:, 0:cT, :],
                            in0=slot_sb[:, base:base + cT, None]
                                .to_broadcast([128, cT, 128]),
                            in1=iota_sb[:, None, :].to_broadcast([128, cT, 128]),
                            op=mybir.AluOpType.is_equal)
                        atmp = tp.tile([128, GCHUNK_TILES, 128], F32, tag="at")
                        nc.vector.tensor_tensor(
                            out=atmp[:, 0:cT, :], in0=m01[:, 0:cT, :],
                            in1=adrep[:, None, :].to_broadcast([128, cT, 128]),
                            op=mybir.AluOpType.mult)
                        ecol = tp.tile([128, GCHUNK_TILES, 1], F32, tag="ec")
                        nc.vector.tensor_reduce(
                            out=ecol[:, 0:cT, :], in_=atmp[:, 0:cT, :],
                            axis=mybir.AxisListType.X, op=mybir.AluOpType.add)
                        nc.vector.tensor_tensor(
                            out=ecol[:, 0:cT, :], in0=ecol[:, 0:cT, :],
                            in1=g_t[:, 0:cT, FD + 1:FD + 2],
                            op=mybir.AluOpType.add)
                        xcol = tp.tile([128, GCHUNK_TILES, 1], F32, tag="xc")
                        nc.vector.tensor_scalar_mul(
                            xcol[:, 0:cT, :], ecol[:, 0:cT, :], SLOPE_GAT)
                        nc.vector.tensor_tensor(
                            out=xcol[:, 0:cT, :], in0=ecol[:, 0:cT, :],
                            in1=xcol[:, 0:cT, :], op=mybir.AluOpType.max)
                        nc.scalar.activation(
                            out=xcol[:, 0:cT, :], in_=xcol[:, 0:cT, :],
                            func=mybir.ActivationFunctionType.Exp)
                        rhsw = tp.tile([128, GCHUNK_TILES, FD + 1], F32, tag="rw")
                        nc.vector.tensor_tensor(
                            out=rhsw[:, 0:cT, :], in0=g_t[:, 0:cT, 0:FD + 1],
                            in1=xcol[:, 0:cT, :].to_broadcast(
                                [128, cT, FD + 1]),
                            op=mybir.AluOpType.mult)
                        for t in range(cT):
                            nc.tensor.matmul(
                                out=psw[:], lhsT=m01[:, t, :],
                                rhs=rhsw[:, t, :],
                                start=(mm_i == 0), stop=(mm_i == n_mm - 1))
                            mm_i += 1
                # post: normalize window
                den = pp.tile([128, 1], F32, tag="den")
                nc.scalar.activation(out=den[:], in_=psw[:, FD:FD + 1],
                                     func=mybir.ActivationFunctionType.Copy,
                                     bias=1e-30)
                nc.vector.reciprocal(out=den[:], in_=den[:])
                outw = pp.tile([128, FD], F32, tag="outw")
                nc.vector.tensor_tensor(
                    out=outw[:], in0=psw[:, 0:FD],
                    in1=den[:].to_broadcast([128, FD]),
                    op=mybir.AluOpType.mult)
                nc.vector.tensor_tensor(out=outw[:], in0=outw[:], in1=b_sb[:],
                                        op=mybir.AluOpType.add)
                if relu_out:
                    nc.scalar.activation(
                        out=xz_sb[:, w * FD:(w + 1) * FD], in_=outw[:],
                        func=mybir.ActivationFunctionType.Relu)
                else:
                    nc.vector.tensor_copy(
                        out=xz_sb[:, w * FD:(w + 1) * FD], in_=outw[:])
            nc.sync.dma_start(out=xz_out[g][:], in_=xz_sb[:])

            if with_tprob and g == "r":
                for w0 in range(0, NW, 4):
                    nw_c = min(4, NW - w0)
                    cw = nw_c * 128
                    xzT = pp.tile([64, 512], F32, tag="xzT")
                    for j in range(nw_c):
                        tps_ = mps.tile([64, 128], F32, space="PSUM", tag="tT")
                        nc.tensor.transpose(
                            out=tps_[:],
                            in_=xz_sb[:, (w0 + j) * FD:(w0 + j + 1) * FD],
                            identity=ident[:])
                        nc.vector.tensor_copy(
                            out=xzT[:, j * 128:(j + 1) * 128], in_=tps_[:])
                    tpp = mps.tile([2, 512], F32, space="PSUM", tag="tpp")
                    nc.tensor.matmul(out=tpp[:, 0:cw], lhsT=wp_sb[:],
                                     rhs=xzT[:, 0:cw], start=True, stop=True)
                    tps = pp.tile([2, 512], F32, tag="tps")
                    nc.vector.tensor_tensor(
                        out=tps[:, 0:cw], in0=tpp[:2, 0:cw],
                        in1=bp_sb[:2, :].to_broadcast([2, cw]),
                        op=mybir.AluOpType.add)
                    nc.sync.dma_start(
                        out=tp_out[:, w0 * 128:w0 * 128 + cw],
                        in_=tps[:, 0:cw])

    nc.compile()
    return nc


def build_heads_launch():
    """Launch C: 4 streams of host-gathered, host-transposed rows -> MLPs.

    Per stream s: in sT [64, HEAD_PAD]; out yT [1, HEAD_PAD].
    nets: (s0->Wy1), (s1->Wy0), (s2->Wy0), (s3->Wy1)
    """
    nc = bacc.Bacc("TRN2", target_bir_lowering=False)
    ins = [nc.dram_tensor(f"s{i}", [FD, HEAD_PAD], F32, kind="ExternalInput")
           for i in range(4)]
    wys = nc.dram_tensor("WyS", [FD, FD], F32, kind="ExternalInput")
    bys = nc.dram_tensor("byS", [128, 1], F32, kind="ExternalInput")
    wnets = {}
    for nm in ("1", "0"):
        wnets[nm] = (
            nc.dram_tensor(f"Wy{nm}a", [FD, FD], F32, kind="ExternalInput"),
            nc.dram_tensor(f"by{nm}a", [128, 1], F32, kind="ExternalInput"),
            nc.dram_tensor(f"Wy{nm}b", [FD, 1], F32, kind="ExternalInput"),
            nc.dram_tensor(f"by{nm}b", [128, 1], F32, kind="ExternalInput"),
        )
    outs = [nc.dram_tensor(f"y{i}", [1, HEAD_PAD], F32, kind="ExternalOutput")
            for i in range(4)]
    net_of = ["1", "0", "0", "1"]

    with tile.TileContext(nc) as tc:
        with tc.tile_pool(name="c", bufs=1) as cp, \
             tc.tile_pool(name="s", bufs=3) as sp, \
             tc.tile_pool(name="ps", bufs=2, space="PSUM") as ps:
            wys_sb = cp.tile([FD, FD], F32)
            nc.sync.dma_start(out=wys_sb[:], in_=wys[:])
            bys_sb = cp.tile([128, 1], F32)
            nc.sync.dma_start(out=bys_sb[:], in_=bys[:])
            wsb = {}
            for nm in ("1", "0"):
                wa, ba, wb, bb = wnets[nm]
                wa_sb = cp.tile([FD, FD], F32, tag=f"wa{nm}")
                nc.sync.dma_start(out=wa_sb[:], in_=wa[:])
                ba_sb = cp.tile([128, 1], F32, tag=f"ba{nm}")
                nc.sync.dma_start(out=ba_sb[:], in_=ba[:])
                wb_sb = cp.tile([FD, 1], F32, tag=f"wb{nm}")
                nc.sync.dma_start(out=wb_sb[:], in_=wb[:])
                bb_sb = cp.tile([128, 1], F32, tag=f"bb{nm}")
                nc.sync.dma_start(out=bb_sb[:], in_=bb[:])
                wsb[nm] = (wa_sb, ba_sb, wb_sb, bb_sb)
            for i in range(4):
                wa_sb, ba_sb, wb_sb, bb_sb = wsb[net_of[i]]
                gin = sp.tile([FD, HEAD_PAD], F32, tag="gin")
                nc.sync.dma_start(out=gin[:], in_=ins[i][:])
                yrow = sp.tile([1, HEAD_PAD], F32, tag="yrow")
                for c0 in range(0, HEAD_PAD, 512):
                    cw = min(512, HEAD_PAD - c0)
                    p1 = ps.tile([FD, 512], F32, space="PSUM", tag="p1")
                    nc.tensor.matmul(out=p1[:, 0:cw], lhsT=wys_sb[:],
                                     rhs=gin[:, c0:c0 + cw],
                                     start=True, stop=True)
                    s1 = sp.tile([FD, 512], F32, tag="s1")
                    t1 = sp.tile([FD, 512], F32, tag="t1")
                    nc.vector.tensor_tensor(
                        out=s1[:, 0:cw], in0=p1[:, 0:cw],
                        in1=bys_sb[:FD, :].to_broadcast([FD, cw]),
                        op=mybir.AluOpType.add)
                    nc.vector.tensor_scalar_mul(
                        t1[:, 0:cw], s1[:, 0:cw], SLOPE_MLP)
                    nc.vector.tensor_tensor(
                        out=s1[:, 0:cw], in0=s1[:, 0:cw], in1=t1[:, 0:cw],
                        op=mybir.AluOpType.max)
                    p2 = ps.tile([FD, 512], F32, space="PSUM", tag="p2")
                    nc.tensor.matmul(out=p2[:, 0:cw], lhsT=wa_sb[:],
                                     rhs=s1[:, 0:cw], start=True, stop=True)
                    s2 = sp.tile([FD, 512], F32, tag="s2")
                    t2 = sp.tile([FD, 512], F32, tag="t2")
                    nc.vector.tensor_tensor(
                        out=s2[:, 0:cw], in0=p2[:, 0:cw],
                        in1=ba_sb[:FD, :].to_broadcast([FD, cw]),
                        op=mybir.AluOpType.add)
                    nc.vector.tensor_scalar_mul(
                        t2[:, 0:cw], s2[:, 0:cw], SLOPE_MLP)
                    nc.vector.tensor_tensor(
                        out=s2[:, 0:cw], in0=s2[:, 0:cw], in1=t2[:, 0:cw],
                        op=mybir.AluOpType.max)
                    p3 = ps.tile([1, 512], F32, space="PSUM", tag="p3")
                    nc.tensor.matmul(out=p3[:, 0:cw], lhsT=wb_sb[:],
                                     rhs=s2[:, 0:cw], start=True, stop=True)
                    s3 = sp.tile([1, 512], F32, tag="s3")
                    t3 = sp.tile([1, 512], F32, tag="t3")
                    nc.vector.tensor_tensor(
                        out=s3[:, 0:cw], in0=p3[:1, 0:cw],
                        in1=bb_sb[:1, :].to_broadcast([1, cw]),
                        op=mybir.AluOpType.add)
                    nc.vector.tensor_scalar_mul(
                        t3[:, 0:cw], s3[:, 0:cw], SLOPE_MLP)
                    nc.vector.tensor_tensor(
                        out=yrow[:, c0:c0 + cw], in0=s3[:, 0:cw],
                        in1=t3[:, 0:cw], op=mybir.AluOpType.max)
                nc.sync.dma_start(out=outs[i][:], in_=yrow[:])
    nc.compile()
    return nc


# ----------------------------------------------------------------- plumbing

def _repl_row(v, rows=128):
    v = np.asarray(v, dtype=np.float32).reshape(1, -1)
    return np.repeat(v, rows, axis=0).copy()


def _col(v):
    out = np.zeros((128, 1), dtype=np.float32)
    a = np.asarray(v, dtype=np.float32).ravel()
    out[:a.shape[0], 0] = a
    return out


def _pad_xT(x):
    """[n, d] -> padded transposed [d, NPAD] f32."""
    x = np.asarray(x, dtype=np.float32)
    out = np.zeros((x.shape[1], NPAD), dtype=np.float32)
    out[:, :x.shape[0]] = x.T
    return out


def _assemble(xz_list):
    """8x [128, NW*64] -> [NPAD, 64]."""
    full = np.zeros((NPAD, FD), dtype=np.float32)
    for k, a in enumerate(xz_list):
        blk = a.reshape(128, NW, FD).transpose(1, 0, 2).reshape(NOWN, FD)
        full[k * NOWN:(k + 1) * NOWN] = blk
    return full


def _run(nc, in_maps, label):
    res = run_bass_kernel_spmd(nc, in_maps, core_ids=list(range(NCORES)),
                               trace=_TRACE)
    if res.exec_time_ns is not None:
        LAST_EXEC_NS.append((label, res.exec_time_ns))
    return res.results


_iota = np.tile(np.arange(128, dtype=np.float32), (128, 1)).copy()


def kernel(x, edge_index, fake_x, fake_edge_index, treat_idx, control_idx,
           W1, as1, ad1, b1, W2, as2, ad2, b2,
           WyS, byS, Wy1a, by1a, Wy1b, by1b, Wy0a, by0a, Wy0b, by0b, Wp, bp):
    LAST_EXEC_NS.clear()
    meta_r, cores_r = prep_graph(edge_index)
    meta_f, cores_f = prep_graph(fake_edge_index)

    def stream_maps():
        maps = []
        for k in range(NCORES):
            m = {}
            for g, cs in (("r", cores_r), ("f", cores_f)):
                for key in ("idx_lo", "slot_lo", "idx_hi", "slot_hi",
                            "own_lo", "own_hi"):
                    m[f"{key}_{g}"] = cs[k][key]
            maps.append(m)
        return maps

    smaps = stream_maps()

    # ---- launch A: conv1
    ncA = build_conv_launch(IN_DIM, [meta_r, meta_f], relu_out=True,
                            with_tprob=False)
    common_a = {
        "xT_r": _pad_xT(x), "xT_f": _pad_xT(fake_x),
        "W": np.asarray(W1, dtype=np.float32),
        "as_r": _repl_row(as1), "ad_r": _repl_row(ad1), "b_r": _repl_row(b1),
        "iota": _iota,
    }
    in_maps = [{**common_a, **smaps[k]} for k in range(NCORES)]
    resA = _run(ncA, in_maps, "conv1")
    xz1 = _assemble([resA[k]["xz_r"] for k in range(NCORES)])
    xfz1 = _assemble([resA[k]["xz_f"] for k in range(NCORES)])

    # ---- launch B: conv2 + tprob
    ncB = build_conv_launch(FD, [meta_r, meta_f], relu_out=False,
                            with_tprob=True)
    common_b = {
        "xT_r": xz1.T.copy(), "xT_f": xfz1.T.copy(),
        "W": np.asarray(W2, dtype=np.float32),
        "as_r": _repl_row(as2), "ad_r": _repl_row(ad2), "b_r": _repl_row(b2),
        "iota": _iota,
        "Wp": np.asarray(Wp, dtype=np.float32), "bp": _col(bp),
    }
    in_maps = [{**common_b, **smaps[k]} for k in range(NCORES)]
    resB = _run(ncB, in_maps, "conv2")
    xz2 = _assemble([resB[k]["xz_r"] for k in range(NCORES)])
    xfz2 = _assemble([resB[k]["xz_f"] for k in range(NCORES)])
    tprob = np.concatenate(
        [resB[k]["tprobT"].T for k in range(NCORES)], axis=0)[:N_REAL]

    # ---- launch C: heads (host gathers + transposes rows)
    ncC = build_heads_launch()
    ti = np.asarray(treat_idx).astype(np.int64)
    ci = np.asarray(control_idx).astype(np.int64)
    gathered = [xz2[ti], xfz2[ti], xz2[ci], xfz2[ci]]
    per_core = T_CNT // NCORES  # 3125
    in_maps = []
    for k in range(NCORES):
        m = {
            "WyS": np.asarray(WyS, np.float32), "byS": _col(byS),
            "Wy1a": np.asarray(Wy1a, np.float32), "by1a": _col(by1a),
            "Wy1b": np.asarray(Wy1b, np.float32), "by1b": _col(by1b),
            "Wy0a": np.asarray(Wy0a, np.float32), "by0a": _col(by0a),
            "Wy0b": np.asarray(Wy0b, np.float32), "by0b": _col(by0b),
        }
        for i in range(4):
            sl = gathered[i][k * per_core:(k + 1) * per_core]
            buf = np.zeros((FD, HEAD_PAD), dtype=np.float32)
            buf[:, :sl.shape[0]] = sl.T
            m[f"s{i}"] = buf
        in_maps.append(m)
    resC = _run(ncC, in_maps, "heads")
    ys = []
    for i in range(4):
        ys.append(np.concatenate(
            [resC[k][f"y{i}"][0, :per_core] for k in range(NCORES)]))
    y1, yc0, y0, yc1 = ys

    return (y1, yc0, y0, yc1, xz2[:N_REAL].copy(), xfz2[:N_REAL].copy(),
            tprob)


# revision 10
# speedup vs baseline: 1.0177x; 1.0177x over previous
"""Self-contained Trainium2 kernel for the BotImpact GNN (2x GATConv on two
graphs + MLP heads), SPMD across 8 NeuronCores.

Strategy (1D node sharding):
 - core k owns nodes [k*6272, (k+1)*6272) of the zero-padded node space
   (N padded 50000 -> 50176 so every core owns exactly 49 windows of 128).
 - Per conv: dense phase computes H rows [h(64) | 1.0 | a_s | a_d | pad] for
   ALL nodes on every core (replicated compute, inputs host-transposed);
   edge phase gathers H[src] rows per edge with dma_gather (int16 indices ->
   table split in two halves at row 25088, each with a trailing zero pad row)
   and aggregates the segment softmax per 128-dst-node window with selection
   -matrix matmuls on the TensorEngine accumulating in PSUM:
       out[slot,:] = sum_e M01[e,slot] * (exp(lrelu(a_s[src]+a_d[dst])) * [h|1])
   then out = num/den (softmax denominator = the appended ones column).
 - Halo exchange between convs is done on the host (3 launches):
   A: conv1 (both graphs) -> B: conv2 + tprob -> C: heads (host pre-gathers
   treat/control rows; device runs the MLPs).
"""

import os
import numpy as np

import concourse.bacc as bacc
import concourse.bass as bass
import concourse.mybir as mybir
import concourse.tile as tile
from concourse.bass_utils import run_bass_kernel_spmd
from concourse.masks import make_identity

F32 = mybir.dt.float32
I16 = mybir.dt.int16

NCORES = 8
N_REAL = 50000
NPAD = 50176            # 392 tiles of 128
NOWN = NPAD // NCORES   # 6272
NW = NOWN // 128        # 49 windows per core
NTILE = NPAD // 128     # 392
HALF = NPAD // 2        # 25088  (= 49*512, quad aligned)
ROW = 128               # H row width (f32) = 512B
FD = 64                 # feature dim
IN_DIM = 128
T_CNT = 25000
HEAD_PAD = ((T_CNT // NCORES) + 127) // 128 * 128  # 3200 per core
SLOPE_GAT = 0.2
SLOPE_MLP = 0.01
GCHUNK_TILES = 8        # <=1024 idx per dma_gather

# H table rows: [0,25088) lo | 25088 zero | [25089,50177) hi | 50177 zero
HROWS = NPAD + 2
LO_DUMMY = HALF         # idx 25088 -> zero row of lo table
HI_DUMMY = HALF         # idx 25088 -> zero row of hi table (row 50177)

LAST_EXEC_NS = []       # per-launch HW exec times (filled when tracing)
_TRACE = bool(int(os.environ.get("KERNEL_TRACE", "0")))


# ----------------------------------------------------------------- host prep

def _pack_idx(idx_tiles):
    """[T,128] int array -> [128, T*8] int16 wrapped mod 16, replicated x8."""
    t = idx_tiles.shape[0]
    out = np.zeros((128, t * 8), dtype=np.int16)
    for ti in range(t):
        w = idx_tiles[ti].reshape(8, 16).T.astype(np.int16)  # [16, 8]
        out[:, ti * 8:(ti + 1) * 8] = np.tile(w, (8, 1))
    return out


def prep_graph(edge_index):
    """Returns (meta, per_core_arrays).

    meta: dict with 'ct' [NW,2] common tile counts per (window, half).
    per core: idx_lo [128,TLlo*8] i16, slot_lo [128,TLlo] f32, idx_hi,
    slot_hi, own_lo [128,NW*8] i16, own_hi [128,NW*8] i16.
    """
    ei = np.asarray(edge_index).astype(np.int64)
    loop = np.arange(N_REAL, dtype=np.int64)
    src = np.concatenate([ei[0], loop])
    dst = np.concatenate([ei[1], loop])

    per_core_edges = []
    counts = np.zeros((NCORES, NW, 2), dtype=np.int64)
    for k in range(NCORES):
        m = (dst // NOWN) == k
        s, d = src[m], dst[m] - k * NOWN
        w = d >> 7
        slot = d & 127
        half = (s >= HALF).astype(np.int64)
        order = np.lexsort((half, w))
        s, w, slot, half = s[order], w[order], slot[order], half[order]
        per_core_edges.append((s, w, slot, half))
        for wi in range(NW):
            wm = w == wi
            counts[k, wi, 0] = np.count_nonzero(wm & (half == 0))
            counts[k, wi, 1] = np.count_nonzero(wm & (half == 1))

    ct = np.zeros((NW, 2), dtype=np.int64)
    for wi in range(NW):
        for h in range(2):
            ct[wi, h] = -(-counts[:, wi, h].max() // 128)  # ceil

    tl_lo, tl_hi = int(ct[:, 0].sum()), int(ct[:, 1].sum())
    cores = []
    for k in range(NCORES):
        s, w, slot, half = per_core_edges[k]
        idx_lo = np.full((tl_lo * 128,), LO_DUMMY, dtype=np.int64)
        slot_lo = np.full((tl_lo * 128,), -1.0, dtype=np.float32)
        idx_hi = np.full((tl_hi * 128,), HI_DUMMY, dtype=np.int64)
        slot_hi = np.full((tl_hi * 128,), -1.0, dtype=np.float32)
        off = {0: 0, 1: 0}
        for wi in range(NW):
            for h, (idx_a, slot_a) in ((0, (idx_lo, slot_lo)),
                                       (1, (idx_hi, slot_hi))):
                m = (w == wi) & (half == h)
                se = s[m] - (HALF if h else 0)
                sl = slot[m].astype(np.float32)
                n = se.shape[0]
                idx_a[off[h]:off[h] + n] = se
                slot_a[off[h]:off[h] + n] = sl
                off[h] += int(ct[wi, h]) * 128
        # own-node gathers: window wi -> global ids k*NOWN + 128*wi + p
        own_lo = np.full((NW, 128), LO_DUMMY, dtype=np.int64)
        own_hi = np.full((NW, 128), HI_DUMMY, dtype=np.int64)
        for wi in range(NW):
            ids = k * NOWN + 128 * wi + np.arange(128)
            lo_m = ids < HALF
            own_lo[wi, lo_m] = ids[lo_m]
            own_hi[wi, ~lo_m] = ids[~lo_m] - HALF
        cores.append({
            "idx_lo": _pack_idx(idx_lo.reshape(tl_lo, 128)),
            "slot_lo": slot_lo.reshape(tl_lo, 128).T.copy(),  # [128, TLlo]
            "idx_hi": _pack_idx(idx_hi.reshape(tl_hi, 128)),
            "slot_hi": slot_hi.reshape(tl_hi, 128).T.copy(),
            "own_lo": _pack_idx(own_lo),
            "own_hi": _pack_idx(own_hi),
        })
    meta = {"ct": ct, "tl_lo": tl_lo, "tl_hi": tl_hi}
    return meta, cores


# ------------------------------------------------------------- conv builder

def _chunks(n_tiles):
    out = []
    t = 0
    while t < n_tiles:
        c = min(GCHUNK_TILES, n_tiles - t)
        out.append((t, c))
        t += c
    return out


def build_conv_launch(kdim, metas, relu_out, with_tprob):
    """One conv layer applied to both graphs. metas = [meta_r, meta_f].

    Inputs (per core): xT_r [kdim,NPAD], xT_f, W [kdim,64], asr/adr/br
    [128,64] (replicated rows), iota [128,128], per-graph edge streams.
    Outputs: xz_r / xz_f [128, NW*64]  (+ tprobT [2, NOWN] if with_tprob).
    """
    nc = bacc.Bacc("TRN2", target_bir_lowering=False)
    g_names = ["r", "f"]
    xT = {g: nc.dram_tensor(f"xT_{g}", [kdim, NPAD], F32, kind="ExternalInput")
          for g in g_names}
    W_in = nc.dram_tensor("W", [kdim, FD], F32, kind="ExternalInput")
    as_in = nc.dram_tensor("as_r", [128, FD], F32, kind="ExternalInput")
    ad_in = nc.dram_tensor("ad_r", [128, FD], F32, kind="ExternalInput")
    b_in = nc.dram_tensor("b_r", [128, FD], F32, kind="ExternalInput")
    iota_in = nc.dram_tensor("iota", [128, 128], F32, kind="ExternalInput")
    if with_tprob:
        wp_in = nc.dram_tensor("Wp", [FD, 2], F32, kind="ExternalInput")
        bp_in = nc.dram_tensor("bp", [128, 1], F32, kind="ExternalInput")

    streams = {}
    for g, meta in zip(g_names, metas):
        tl_lo, tl_hi = meta["tl_lo"], meta["tl_hi"]
        streams[g] = {
            "idx_lo": nc.dram_tensor(f"idx_lo_{g}", [128, tl_lo * 8], I16,
                                     kind="ExternalInput"),
            "slot_lo": nc.dram_tensor(f"slot_lo_{g}", [128, tl_lo], F32,
                                      kind="ExternalInput"),
            "idx_hi": nc.dram_tensor(f"idx_hi_{g}", [128, tl_hi * 8], I16,
                                     kind="ExternalInput"),
            "slot_hi": nc.dram_tensor(f"slot_hi_{g}", [128, tl_hi], F32,
                                      kind="ExternalInput"),
            "own_lo": nc.dram_tensor(f"own_lo_{g}", [128, NW * 8], I16,
                                     kind="ExternalInput"),
            "own_hi": nc.dram_tensor(f"own_hi_{g}", [128, NW * 8], I16,
                                     kind="ExternalInput"),
        }

    H = {g: nc.dram_tensor(f"H_{g}", [HROWS, ROW], F32) for g in g_names}
    xz_out = {g: nc.dram_tensor(f"xz_{g}", [128, NW * FD], F32,
                                kind="ExternalOutput") for g in g_names}
    if with_tprob:
        tp_out = nc.dram_tensor("tprobT", [2, NOWN], F32, kind="ExternalOutput")

    from contextlib import ExitStack
    with tile.TileContext(nc) as tc, ExitStack() as es:
        cpool = es.enter_context(tc.tile_pool(name="const", bufs=1))
        spool = es.enter_context(tc.tile_pool(name="streams", bufs=1))
        dq = es.enter_context(tc.tile_pool(name="densequad", bufs=3))
        dps = es.enter_context(tc.tile_pool(name="densepsum", bufs=2, space="PSUM"))
        mps = es.enter_context(tc.tile_pool(name="miscpsum", bufs=1, space="PSUM"))
        gp = es.enter_context(tc.tile_pool(name="gtiles", bufs=4))
        mp = es.enter_context(tc.tile_pool(name="m01", bufs=4))
        tp = es.enter_context(tc.tile_pool(name="tmp", bufs=4))
        wp_ = es.enter_context(tc.tile_pool(name="wpsum", bufs=2, space="PSUM"))
        pp = es.enter_context(tc.tile_pool(name="post", bufs=4))
        xp = es.enter_context(tc.tile_pool(name="xzown", bufs=1))

        W_sb = cpool.tile([kdim, FD], F32)
        nc.sync.dma_start(out=W_sb[:], in_=W_in[:])
        as_sb = cpool.tile([128, FD], F32)
        nc.sync.dma_start(out=as_sb[:], in_=as_in[:])
        ad_sb = cpool.tile([128, FD], F32)
        nc.sync.dma_start(out=ad_sb[:], in_=ad_in[:])
        b_sb = cpool.tile([128, FD], F32)
        nc.sync.dma_start(out=b_sb[:], in_=b_in[:])
        iota_sb = cpool.tile([128, 128], F32)
        nc.sync.dma_start(out=iota_sb[:], in_=iota_in[:])
        ones_row = cpool.tile([1, 128], F32)
        nc.vector.memset(ones_row[:], 1.0)
        ident = cpool.tile([128, 128], F32)
        make_identity(nc, ident[:])
        if with_tprob:
            wp_sb = cpool.tile([FD, 2], F32)
            nc.sync.dma_start(out=wp_sb[:], in_=wp_in[:])
            bp_sb = cpool.tile([128, 1], F32)
            nc.sync.dma_start(out=bp_sb[:], in_=bp_in[:])

        # zero the two pad rows of each H table
        for g in g_names:
            zrow = cpool.tile([1, ROW], F32, tag="zrow")
            nc.vector.memset(zrow[:], 0.0)
            nc.sync.dma_start(out=H[g][HALF:HALF + 1, :], in_=zrow[:])
            nc.sync.dma_start(out=H[g][HROWS - 1:HROWS, :], in_=zrow[:])

        for g, meta in zip(g_names, metas):
            st = streams[g]
            tl_lo, tl_hi = meta["tl_lo"], meta["tl_hi"]
            ct = meta["ct"]

            idx_lo_sb = spool.tile([128, tl_lo * 8], I16, tag=f"il{g}")
            nc.sync.dma_start(out=idx_lo_sb[:], in_=st["idx_lo"][:])
            slot_lo_sb = spool.tile([128, tl_lo], F32, tag=f"sl{g}")
            nc.sync.dma_start(out=slot_lo_sb[:], in_=st["slot_lo"][:])
            idx_hi_sb = spool.tile([128, tl_hi * 8], I16, tag=f"ih{g}")
            nc.sync.dma_start(out=idx_hi_sb[:], in_=st["idx_hi"][:])
            slot_hi_sb = spool.tile([128, tl_hi], F32, tag=f"sh{g}")
            nc.sync.dma_start(out=slot_hi_sb[:], in_=st["slot_hi"][:])
            own_lo_sb = spool.tile([128, NW * 8], I16, tag=f"ol{g}")
            nc.sync.dma_start(out=own_lo_sb[:], in_=st["own_lo"][:])
            own_hi_sb = spool.tile([128, NW * 8], I16, tag=f"oh{g}")
            nc.sync.dma_start(out=own_hi_sb[:], in_=st["own_hi"][:])

            # ---------------- dense: H rows for all nodes (quads of 4 tiles)
            for q in range(NTILE // 4):
                lt = dq.tile([kdim, 512], F32, tag="lhs")
                nc.sync.dma_start(out=lt[:], in_=xT[g][:, q * 512:(q + 1) * 512])
                hp = dps.tile([128, 4, FD], F32, space="PSUM", tag="hps")
                for j in range(4):
                    nc.tensor.matmul(
                        out=hp[:, j, :], lhsT=lt[:, j * 128:(j + 1) * 128],
                        rhs=W_sb[:], start=True, stop=True)
                hq = dq.tile([128, 4, ROW], F32, tag="hq")
                nc.vector.memset(hq[:], 0.0)
                nc.vector.tensor_copy(out=hq[:, :, 0:FD], in_=hp[:])
                nc.vector.memset(hq[:, :, FD:FD + 1], 1.0)
                tmp = dq.tile([128, 4, FD], F32, tag="dtmp")
                nc.vector.tensor_tensor(
                    out=tmp[:], in0=hq[:, :, 0:FD],
                    in1=as_sb[:, None, :].to_broadcast([128, 4, FD]),
                    op=mybir.AluOpType.mult)
                nc.vector.tensor_reduce(
                    out=hq[:, :, FD + 1:FD + 2], in_=tmp[:],
                    axis=mybir.AxisListType.X, op=mybir.AluOpType.add)
                nc.vector.tensor_tensor(
                    out=tmp[:], in0=hq[:, :, 0:FD],
                    in1=ad_sb[:, None, :].to_broadcast([128, 4, FD]),
                    op=mybir.AluOpType.mult)
                nc.vector.tensor_reduce(
                    out=hq[:, :, FD + 2:FD + 3], in_=tmp[:],
                    axis=mybir.AxisListType.X, op=mybir.AluOpType.add)
                row0 = q * 512 + (1 if q * 512 >= HALF else 0)
                nc.sync.dma_start(
                    out=H[g][row0:row0 + 512, :].rearrange(
                        "(t p) r -> p t r", p=128),
                    in_=hq[:])

            h_lo = H[g][0:HALF + 1, :]
            h_hi = H[g][HALF + 1:HROWS, :]

            # ---------------- edge phase
            xz_sb = xp.tile([128, NW * FD], F32, tag=f"xz{g}")
            lo_off = np.concatenate([[0], np.cumsum(ct[:, 0])]).astype(int)
            hi_off = np.concatenate([[0], np.cumsum(ct[:, 1])]).astype(int)
            for w in range(NW):
                # own-node rows -> a_d column for this window
                go_l = gp.tile([128, 1, ROW], F32, tag="gown")
                nc.gpsimd.dma_gather(
                    go_l[:], h_lo[:], own_lo_sb[:, w * 8:(w + 1) * 8],
                    128, 128, ROW)
                go_h = gp.tile([128, 1, ROW], F32, tag="gown")
                nc.gpsimd.dma_gather(
                    go_h[:], h_hi[:], own_hi_sb[:, w * 8:(w + 1) * 8],
                    128, 128, ROW)
                adcol = pp.tile([128, 1], F32, tag="adcol")
                nc.vector.tensor_tensor(
                    out=adcol[:], in0=go_l[:, 0, FD + 2:FD + 3],
                    in1=go_h[:, 0, FD + 2:FD + 3], op=mybir.AluOpType.add)
                # transpose [128,1] -> [1,128], then K=1 matmul to replicate
                adT = mps.tile([1, 128], F32, space="PSUM", tag="adT")
                nc.tensor.transpose(out=adT[:], in_=adcol[:], identity=ident[:])
                adT_sb = pp.tile([1, 128], F32, tag="adTs")
                nc.vector.tensor_copy(out=adT_sb[:], in_=adT[:])
                adrep_ps = mps.tile([128, 128], F32, space="PSUM", tag="adrep")
                nc.tensor.matmul(out=adrep_ps[:], lhsT=ones_row[:],
                                 rhs=adT_sb[:], start=True, stop=True)
                adrep = pp.tile([128, 128], F32, tag="adrep_sb")
                nc.vector.tensor_copy(out=adrep[:], in_=adrep_ps[:])

                psw = wp_.tile([128, FD + 1], F32, space="PSUM", tag="psw")
                n_mm = int(ct[w, 0] + ct[w, 1])
                mm_i = 0
                for h_ix, (idx_sb, slot_sb, tbl, off_a) in enumerate(
                        ((idx_lo_sb, slot_lo_sb, h_lo, lo_off),
                         (idx_hi_sb, slot_hi_sb, h_hi, hi_off))):
                    for (t0, cT) in _chunks(int(ct[w, h_ix])):
                        base = int(off_a[w]) + t0
                        g_t = gp.tile([128, GCHUNK_TILES, ROW], F32, tag="g")
                        nc.gpsimd.dma_gather(
                            g_t[:, 0:cT, :], tbl[:],
                            idx_sb[:, base * 8:(base + cT) * 8],
                            cT * 128, cT * 128, ROW)
                        m01 = mp.tile([128, GCHUNK_TILES, 128], F32, tag="m")
                        nc.vector.tensor_tensor(
                            out=m01[:, 0:cT, :],
                            in0=slot_sb[:, base:base + cT, None]
                                .to_broadcast([128, cT, 128]),
                            in1=iota_sb[:, None, :].to_broadcast([128, cT, 128]),
                            op=mybir.AluOpType.is_equal)
                        atmp = tp.tile([128, GCHUNK_TILES, 128], F32, tag="at")
                        nc.vector.tensor_tensor(
                            out=atmp[:, 0:cT, :], in0=m01[:, 0:cT, :],
                            in1=adrep[:, None, :].to_broadcast([128, cT, 128]),
                            op=mybir.AluOpType.mult)
                        ecol = tp.tile([128, GCHUNK_TILES, 1], F32, tag="ec")
                        nc.vector.tensor_reduce(
                            out=ecol[:, 0:cT, :], in_=atmp[:, 0:cT, :],
                            axis=mybir.AxisListType.X, op=mybir.AluOpType.add)
                        nc.vector.tensor_tensor(
                            out=ecol[:, 0:cT, :], in0=ecol[:, 0:cT, :],
                            in1=g_t[:, 0:cT, FD + 1:FD + 2],
                            op=mybir.AluOpType.add)
                        xcol = tp.tile([128, GCHUNK_TILES, 1], F32, tag="xc")
                        nc.vector.tensor_scalar_mul(
                            xcol[:, 0:cT, :], ecol[:, 0:cT, :], SLOPE_GAT)
                        nc.vector.tensor_tensor(
                            out=xcol[:, 0:cT, :], in0=ecol[:, 0:cT, :],
                            in1=xcol[:, 0:cT, :], op=mybir.AluOpType.max)
                        nc.scalar.activation(
                            out=xcol[:, 0:cT, :], in_=xcol[:, 0:cT, :],
                            func=mybir.ActivationFunctionType.Exp)
                        rhsw = tp.tile([128, GCHUNK_TILES, FD + 1], F32, tag="rw")
                        nc.vector.tensor_tensor(
                            out=rhsw[:, 0:cT, :], in0=g_t[:, 0:cT, 0:FD + 1],
                            in1=xcol[:, 0:cT, :].to_broadcast(
                                [128, cT, FD + 1]),
                            op=mybir.AluOpType.mult)
                        for t in range(cT):
                            nc.tensor.matmul(
                                out=psw[:], lhsT=m01[:, t, :],
                                rhs=rhsw[:, t, :],
                                start=(mm_i == 0), stop=(mm_i == n_mm - 1))
                            mm_i += 1
                # post: normalize window
                den = pp.tile([128, 1], F32, tag="den")
                nc.scalar.activation(out=den[:], in_=psw[:, FD:FD + 1],
                                     func=mybir.ActivationFunctionType.Copy,
                                     bias=1e-30)
                nc.vector.reciprocal(out=den[:], in_=den[:])
                outw = pp.tile([128, FD], F32, tag="outw")
                nc.vector.tensor_tensor(
                    out=outw[:], in0=psw[:, 0:FD],
                    in1=den[:].to_broadcast([128, FD]),
                    op=mybir.AluOpType.mult)
                nc.vector.tensor_tensor(out=outw[:], in0=outw[:], in1=b_sb[:],
                                        op=mybir.AluOpType.add)
                if relu_out:
                    nc.scalar.activation(
                        out=xz_sb[:, w * FD:(w + 1) * FD], in_=outw[:],
                        func=mybir.ActivationFunctionType.Relu)
                else:
                    nc.vector.tensor_copy(
                        out=xz_sb[:, w * FD:(w + 1) * FD], in_=outw[:])
            nc.sync.dma_start(out=xz_out[g][:], in_=xz_sb[:])

            if with_tprob and g == "r":
                for w0 in range(0, NW, 4):
                    nw_c = min(4, NW - w0)
                    cw = nw_c * 128
                    xzT = pp.tile([64, 512], F32, tag="xzT")
                    for j in range(nw_c):
                        tps_ = mps.tile([64, 128], F32, space="PSUM", tag="tT")
                        nc.tensor.transpose(
                            out=tps_[:],
                            in_=xz_sb[:, (w0 + j) * FD:(w0 + j + 1) * FD],
                            identity=ident[:])
                        nc.vector.tensor_copy(
                            out=xzT[:, j * 128:(j + 1) * 128], in_=tps_[:])
                    tpp = mps.tile([2, 512], F32, space="PSUM", tag="tpp")
                    nc.tensor.matmul(out=tpp[:, 0:cw], lhsT=wp_sb[:],
                                     rhs=xzT[:, 0:cw], start=True, stop=True)
                    tps = pp.tile([2, 512], F32, tag="tps")
                    nc.vector.tensor_tensor(
                        out=tps[:, 0:cw], in0=tpp[:2, 0:cw],
                        in1=bp_sb[:2, :].to_broadcast([2, cw]),
                        op=mybir.AluOpType.add)
                    nc.sync.dma_start(
                        out=tp_out[:, w0 * 128:w0 * 128 + cw],
                        in_=tps[:, 0:cw])

    nc.compile()
    return nc


def build_heads_launch():
    """Launch C: 4 streams of host-gathered, host-transposed rows -> MLPs.

    Per stream s: in sT [64, HEAD_PAD]; out yT [1, HEAD_PAD].
    nets: (s0->Wy1), (s1->Wy0), (s2->Wy0), (s3->Wy1)
    """
    nc = bacc.Bacc("TRN2", target_bir_lowering=False)
    ins = [nc.dram_tensor(f"s{i}", [FD, HEAD_PAD], F32, kind="ExternalInput")
           for i in range(4)]
    wys = nc.dram_tensor("WyS", [FD, FD], F32, kind="ExternalInput")
    bys = nc.dram_tensor("byS", [128, 1], F32, kind="ExternalInput")
    wnets = {}
    for nm in ("1", "0"):
        wnets[nm] = (
            nc.dram_tensor(f"Wy{nm}a", [FD, FD], F32, kind="ExternalInput"),
            nc.dram_tensor(f"by{nm}a", [128, 1], F32, kind="ExternalInput"),
            nc.dram_tensor(f"Wy{nm}b", [FD, 1], F32, kind="ExternalInput"),
            nc.dram_tensor(f"by{nm}b", [128, 1], F32, kind="ExternalInput"),
        )
    outs = [nc.dram_tensor(f"y{i}", [1, HEAD_PAD], F32, kind="ExternalOutput")
            for i in range(4)]
    net_of = ["1", "0", "0", "1"]

    with tile.TileContext(nc) as tc:
        with tc.tile_pool(name="c", bufs=1) as cp, \
             tc.tile_pool(name="s", bufs=3) as sp, \
             tc.tile_pool(name="ps", bufs=2, space="PSUM") as ps:
            wys_sb = cp.tile([FD, FD], F32)
            nc.sync.dma_start(out=wys_sb[:], in_=wys[:])
            bys_sb = cp.tile([128, 1], F32)
            nc.sync.dma_start(out=bys_sb[:], in_=bys[:])
            wsb = {}
            for nm in ("1", "0"):
                wa, ba, wb, bb = wnets[nm]
                wa_sb = cp.tile([FD, FD], F32, tag=f"wa{nm}")
                nc.sync.dma_start(out=wa_sb[:], in_=wa[:])
                ba_sb = cp.tile([128, 1], F32, tag=f"ba{nm}")
                nc.sync.dma_start(out=ba_sb[:], in_=ba[:])
                wb_sb = cp.tile([FD, 1], F32, tag=f"wb{nm}")
                nc.sync.dma_start(out=wb_sb[:], in_=wb[:])
                bb_sb = cp.tile([128, 1], F32, tag=f"bb{nm}")
                nc.sync.dma_start(out=bb_sb[:], in_=bb[:])
                wsb[nm] = (wa_sb, ba_sb, wb_sb, bb_sb)
            for i in range(4):
                wa_sb, ba_sb, wb_sb, bb_sb = wsb[net_of[i]]
                gin = sp.tile([FD, HEAD_PAD], F32, tag="gin")
                nc.sync.dma_start(out=gin[:], in_=ins[i][:])
                yrow = sp.tile([1, HEAD_PAD], F32, tag="yrow")
                for c0 in range(0, HEAD_PAD, 512):
                    cw = min(512, HEAD_PAD - c0)
                    p1 = ps.tile([FD, 512], F32, space="PSUM", tag="p1")
                    nc.tensor.matmul(out=p1[:, 0:cw], lhsT=wys_sb[:],
                                     rhs=gin[:, c0:c0 + cw],
                                     start=True, stop=True)
                    s1 = sp.tile([FD, 512], F32, tag="s1")
                    t1 = sp.tile([FD, 512], F32, tag="t1")
                    nc.vector.tensor_tensor(
                        out=s1[:, 0:cw], in0=p1[:, 0:cw],
                        in1=bys_sb[:FD, :].to_broadcast([FD, cw]),
                        op=mybir.AluOpType.add)
                    nc.vector.tensor_scalar_mul(
                        t1[:, 0:cw], s1[:, 0:cw], SLOPE_MLP)
                    nc.vector.tensor_tensor(
                        out=s1[:, 0:cw], in0=s1[:, 0:cw], in1=t1[:, 0:cw],
                        op=mybir.AluOpType.max)
                    p2 = ps.tile([FD, 512], F32, space="PSUM", tag="p2")
                    nc.tensor.matmul(out=p2[:, 0:cw], lhsT=wa_sb[:],
                                     rhs=s1[:, 0:cw], start=True, stop=True)
                    s2 = sp.tile([FD, 512], F32, tag="s2")
                    t2 = sp.tile([FD, 512], F32, tag="t2")
                    nc.vector.tensor_tensor(
                        out=s2[:, 0:cw], in0=p2[:, 0:cw],
                        in1=ba_sb[:FD, :].to_broadcast([FD, cw]),
                        op=mybir.AluOpType.add)
                    nc.vector.tensor_scalar_mul(
                        t2[:, 0:cw], s2[:, 0:cw], SLOPE_MLP)
                    nc.vector.tensor_tensor(
                        out=s2[:, 0:cw], in0=s2[:, 0:cw], in1=t2[:, 0:cw],
                        op=mybir.AluOpType.max)
                    p3 = ps.tile([1, 512], F32, space="PSUM", tag="p3")
                    nc.tensor.matmul(out=p3[:, 0:cw], lhsT=wb_sb[:],
                                     rhs=s2[:, 0:cw], start=True, stop=True)
                    s3 = sp.tile([1, 512], F32, tag="s3")
                    t3 = sp.tile([1, 512], F32, tag="t3")
                    nc.vector.tensor_tensor(
                        out=s3[:, 0:cw], in0=p3[:1, 0:cw],
                        in1=bb_sb[:1, :].to_broadcast([1, cw]),
                        op=mybir.AluOpType.add)
                    nc.vector.tensor_scalar_mul(
                        t3[:, 0:cw], s3[:, 0:cw], SLOPE_MLP)
                    nc.vector.tensor_tensor(
                        out=yrow[:, c0:c0 + cw], in0=s3[:, 0:cw],
                        in1=t3[:, 0:cw], op=mybir.AluOpType.max)
                nc.sync.dma_start(out=outs[i][:], in_=yrow[:])
    nc.compile()
    return nc


# ----------------------------------------------------------------- plumbing

def _repl_row(v, rows=128):
    v = np.asarray(v, dtype=np.float32).reshape(1, -1)
    return np.repeat(v, rows, axis=0).copy()


def _col(v):
    out = np.zeros((128, 1), dtype=np.float32)
    a = np.asarray(v, dtype=np.float32).ravel()
    out[:a.shape[0], 0] = a
    return out


def _pad_xT(x):
    """[n, d] -> padded transposed [d, NPAD] f32."""
    x = np.asarray(x, dtype=np.float32)
    out = np.zeros((x.shape[1], NPAD), dtype=np.float32)
    out[:, :x.shape[0]] = x.T
    return out


def _assemble(xz_list):
    """8x [128, NW*64] -> [NPAD, 64]."""
    full = np.zeros((NPAD, FD), dtype=np.float32)
    for k, a in enumerate(xz_list):
        blk = a.reshape(128, NW, FD).transpose(1, 0, 2).reshape(NOWN, FD)
        full[k * NOWN:(k + 1) * NOWN] = blk
    return full


def _run(nc, in_maps, label):
    res = run_bass_kernel_spmd(nc, in_maps, core_ids=list(range(NCORES)),
                               trace=_TRACE)
    if res.exec_time_ns is not None:
        LAST_EXEC_NS.append((label, res.exec_time_ns))
    return res.results


_iota = np.tile(np.arange(128, dtype=np.float32), (128, 1)).copy()


def kernel(x, edge_index, fake_x, fake_edge_index, treat_idx, control_idx,
           W1, as1, ad1, b1, W2, as2, ad2, b2,
           WyS, byS, Wy1a, by1a, Wy1b, by1b, Wy0a, by0a, Wy0b, by0b, Wp, bp):
    LAST_EXEC_NS.clear()
    meta_r, cores_r = prep_graph(edge_index)
    meta_f, cores_f = prep_graph(fake_edge_index)

    def stream_maps():
        maps = []
        for k in range(NCORES):
            m = {}
            for g, cs in (("r", cores_r), ("f", cores_f)):
                for key in ("idx_lo", "slot_lo", "idx_hi", "slot_hi",
                            "own_lo", "own_hi"):
                    m[f"{key}_{g}"] = cs[k][key]
            maps.append(m)
        return maps

    smaps = stream_maps()

    # ---- launch A: conv1
    ncA = build_conv_launch(IN_DIM, [meta_r, meta_f], relu_out=True,
                            with_tprob=False)
    common_a = {
        "xT_r": _pad_xT(x), "xT_f": _pad_xT(fake_x),
        "W": np.asarray(W1, dtype=np.float32),
        "as_r": _repl_row(as1), "ad_r": _repl_row(ad1), "b_r": _repl_row(b1),
        "iota": _iota,
    }
    in_maps = [{**common_a, **smaps[k]} for k in range(NCORES)]
    resA = _run(ncA, in_maps, "conv1")
    xz1 = _assemble([resA[k]["xz_r"] for k in range(NCORES)])
    xfz1 = _assemble([resA[k]["xz_f"] for k in range(NCORES)])

    # ---- launch B: conv2 + tprob
    ncB = build_conv_launch(FD, [meta_r, meta_f], relu_out=False,
                            with_tprob=True)
    common_b = {
        "xT_r": xz1.T.copy(), "xT_f": xfz1.T.copy(),
        "W": np.asarray(W2, dtype=np.float32),
        "as_r": _repl_row(as2), "ad_r": _repl_row(ad2), "b_r": _repl_row(b2),
        "iota": _iota,
        "Wp": np.asarray(Wp, dtype=np.float32), "bp": _col(bp),
    }
    in_maps = [{**common_b, **smaps[k]} for k in range(NCORES)]
    resB = _run(ncB, in_maps, "conv2")
    xz2 = _assemble([resB[k]["xz_r"] for k in range(NCORES)])
    xfz2 = _assemble([resB[k]["xz_f"] for k in range(NCORES)])
    tprob = np.concatenate(
        [resB[k]["tprobT"].T for k in range(NCORES)], axis=0)[:N_REAL]

    # ---- launch C: heads (host gathers + transposes rows)
    ncC = build_heads_launch()
    ti = np.asarray(treat_idx).astype(np.int64)
    ci = np.asarray(control_idx).astype(np.int64)
    gathered = [xz2[ti], xfz2[ti], xz2[ci], xfz2[ci]]
    per_core = T_CNT // NCORES  # 3125
    in_maps = []
    for k in range(NCORES):
        m = {
            "WyS": np.asarray(WyS, np.float32), "byS": _col(byS),
            "Wy1a": np.asarray(Wy1a, np.float32), "by1a": _col(by1a),
            "Wy1b": np.asarray(Wy1b, np.float32), "by1b": _col(by1b),
            "Wy0a": np.asarray(Wy0a, np.float32), "by0a": _col(by0a),
            "Wy0b": np.asarray(Wy0b, np.float32), "by0b": _col(by0b),
        }
        for i in range(4):
            sl = gathered[i][k * per_core:(k + 1) * per_core]
            buf = np.zeros((FD, HEAD_PAD), dtype=np.float32)
            buf[:, :sl.shape[0]] = sl.T
            m[f"s{i}"] = buf
        in_maps.append(m)
    resC = _run(ncC, in_maps, "heads")
    ys = []
    for i in range(4):
        ys.append(np.concatenate(
            [resC[k][f"y{i}"][0, :per_core] for k in range(NCORES)]))
    y1, yc0, y0, yc1 = ys

    return (y1, yc0, y0, yc1, xz2[:N_REAL].copy(), xfz2[:N_REAL].copy(),
            tprob)


# revision 11
# speedup vs baseline: 1.0349x; 1.0170x over previous
"""Self-contained Trainium2 kernel for the BotImpact GNN (2x GATConv on two
graphs + MLP heads), SPMD across 8 NeuronCores.

Strategy (1D node sharding):
 - core k owns nodes [k*6272, (k+1)*6272) of the zero-padded node space
   (N padded 50000 -> 50176 so every core owns exactly 49 windows of 128).
 - Per conv: dense phase computes H rows [h(64) | 1.0 | a_s | a_d | pad] for
   ALL nodes on every core (replicated compute, inputs host-transposed);
   edge phase gathers H[src] rows per edge with dma_gather (int16 indices ->
   table split in two halves at row 25088, each with a trailing zero pad row)
   and aggregates the segment softmax per 128-dst-node window with selection
   -matrix matmuls on the TensorEngine accumulating in PSUM:
       out[slot,:] = sum_e M01[e,slot] * (exp(lrelu(a_s[src]+a_d[dst])) * [h|1])
   then out = num/den (softmax denominator = the appended ones column).
 - Halo exchange between convs is done on the host (3 launches):
   A: conv1 (both graphs) -> B: conv2 + tprob -> C: heads (host pre-gathers
   treat/control rows; device runs the MLPs).
"""

import os
import numpy as np

import concourse.bacc as bacc
import concourse.bass as bass
import concourse.mybir as mybir
import concourse.tile as tile
from concourse.bass_utils import run_bass_kernel_spmd
from concourse.masks import make_identity

F32 = mybir.dt.float32
I16 = mybir.dt.int16

NCORES = 8
N_REAL = 50000
NPAD = 50176            # 392 tiles of 128
NOWN = NPAD // NCORES   # 6272
NW = NOWN // 128        # 49 windows per core
NTILE = NPAD // 128     # 392
HALF = NPAD // 2        # 25088  (= 49*512, quad aligned)
ROW = 128               # H row width (f32) = 512B
FD = 64                 # feature dim
IN_DIM = 128
T_CNT = 25000
HEAD_PAD = ((T_CNT // NCORES) + 127) // 128 * 128  # 3200 per core
SLOPE_GAT = 0.2
SLOPE_MLP = 0.01
GCHUNK_TILES = 8        # <=1024 idx per dma_gather

# H table rows: [0,25088) lo | 25088 zero | [25089,50177) hi | 50177 zero
HROWS = NPAD + 2
LO_DUMMY = HALF         # idx 25088 -> zero row of lo table
HI_DUMMY = HALF         # idx 25088 -> zero row of hi table (row 50177)

LAST_EXEC_NS = []       # per-launch HW exec times (filled when tracing)
_TRACE = bool(int(os.environ.get("KERNEL_TRACE", "0")))


# ----------------------------------------------------------------- host prep

def _pack_idx(idx_tiles):
    """[T,128] int array -> [128, T*8] int16 wrapped mod 16, replicated x8."""
    t = idx_tiles.shape[0]
    out = np.zeros((128, t * 8), dtype=np.int16)
    for ti in range(t):
        w = idx_tiles[ti].reshape(8, 16).T.astype(np.int16)  # [16, 8]
        out[:, ti * 8:(ti + 1) * 8] = np.tile(w, (8, 1))
    return out


def prep_graph(edge_index):
    """Returns (meta, per_core_arrays).

    meta: dict with 'ct' [NW,2] common tile counts per (window, half).
    per core: idx_lo [128,TLlo*8] i16, slot_lo [128,TLlo] f32, idx_hi,
    slot_hi, own_lo [128,NW*8] i16, own_hi [128,NW*8] i16.
    """
    ei = np.asarray(edge_index).astype(np.int64)
    loop = np.arange(N_REAL, dtype=np.int64)
    src = np.concatenate([ei[0], loop])
    dst = np.concatenate([ei[1], loop])

    per_core_edges = []
    counts = np.zeros((NCORES, NW, 2), dtype=np.int64)
    for k in range(NCORES):
        m = (dst // NOWN) == k
        s, d = src[m], dst[m] - k * NOWN
        w = d >> 7
        slot = d & 127
        half = (s >= HALF).astype(np.int64)
        order = np.lexsort((half, w))
        s, w, slot, half = s[order], w[order], slot[order], half[order]
        per_core_edges.append((s, w, slot, half))
        for wi in range(NW):
            wm = w == wi
            counts[k, wi, 0] = np.count_nonzero(wm & (half == 0))
            counts[k, wi, 1] = np.count_nonzero(wm & (half == 1))

    ct = np.zeros((NW, 2), dtype=np.int64)
    for wi in range(NW):
        for h in range(2):
            ct[wi, h] = -(-counts[:, wi, h].max() // 128)  # ceil

    tl_lo, tl_hi = int(ct[:, 0].sum()), int(ct[:, 1].sum())
    cores = []
    for k in range(NCORES):
        s, w, slot, half = per_core_edges[k]
        idx_lo = np.full((tl_lo * 128,), LO_DUMMY, dtype=np.int64)
        slot_lo = np.full((tl_lo * 128,), -1.0, dtype=np.float32)
        idx_hi = np.full((tl_hi * 128,), HI_DUMMY, dtype=np.int64)
        slot_hi = np.full((tl_hi * 128,), -1.0, dtype=np.float32)
        off = {0: 0, 1: 0}
        for wi in range(NW):
            for h, (idx_a, slot_a) in ((0, (idx_lo, slot_lo)),
                                       (1, (idx_hi, slot_hi))):
                m = (w == wi) & (half == h)
                se = s[m] - (HALF if h else 0)
                sl = slot[m].astype(np.float32)
                n = se.shape[0]
                idx_a[off[h]:off[h] + n] = se
                slot_a[off[h]:off[h] + n] = sl
                off[h] += int(ct[wi, h]) * 128
        # own-node gathers: window wi -> global ids k*NOWN + 128*wi + p
        own_lo = np.full((NW, 128), LO_DUMMY, dtype=np.int64)
        own_hi = np.full((NW, 128), HI_DUMMY, dtype=np.int64)
        for wi in range(NW):
            ids = k * NOWN + 128 * wi + np.arange(128)
            lo_m = ids < HALF
            own_lo[wi, lo_m] = ids[lo_m]
            own_hi[wi, ~lo_m] = ids[~lo_m] - HALF
        cores.append({
            "idx_lo": _pack_idx(idx_lo.reshape(tl_lo, 128)),
            "slot_lo": slot_lo.reshape(tl_lo, 128).T.copy(),  # [128, TLlo]
            "idx_hi": _pack_idx(idx_hi.reshape(tl_hi, 128)),
            "slot_hi": slot_hi.reshape(tl_hi, 128).T.copy(),
            "own_lo": _pack_idx(own_lo),
            "own_hi": _pack_idx(own_hi),
        })
    meta = {"ct": ct, "tl_lo": tl_lo, "tl_hi": tl_hi}
    return meta, cores


# ------------------------------------------------------------- conv builder

def _chunks(n_tiles):
    out = []
    t = 0
    while t < n_tiles:
        c = min(GCHUNK_TILES, n_tiles - t)
        out.append((t, c))
        t += c
    return out


def build_conv_launch(kdim, metas, relu_out, with_tprob):
    """One conv layer applied to both graphs. metas = [meta_r, meta_f].

    Inputs (per core): xT_r [kdim,NPAD], xT_f, W [kdim,64], asr/adr/br
    [128,64] (replicated rows), iota [128,128], per-graph edge streams.
    Outputs: xz_r / xz_f [128, NW*64]  (+ tprobT [2, NOWN] if with_tprob).
    """
    nc = bacc.Bacc("TRN2", target_bir_lowering=False)
    g_names = ["r", "f"]
    xT = {g: nc.dram_tensor(f"xT_{g}", [kdim, NPAD], F32, kind="ExternalInput")
          for g in g_names}
    W_in = nc.dram_tensor("W", [kdim, FD], F32, kind="ExternalInput")
    as_in = nc.dram_tensor("as_r", [128, FD], F32, kind="ExternalInput")
    ad_in = nc.dram_tensor("ad_r", [128, FD], F32, kind="ExternalInput")
    b_in = nc.dram_tensor("b_r", [128, FD], F32, kind="ExternalInput")
    iota_in = nc.dram_tensor("iota", [128, 128], F32, kind="ExternalInput")
    if with_tprob:
        wp_in = nc.dram_tensor("Wp", [FD, 2], F32, kind="ExternalInput")
        bp_in = nc.dram_tensor("bp", [128, 1], F32, kind="ExternalInput")

    streams = {}
    for g, meta in zip(g_names, metas):
        tl_lo, tl_hi = meta["tl_lo"], meta["tl_hi"]
        streams[g] = {
            "idx_lo": nc.dram_tensor(f"idx_lo_{g}", [128, tl_lo * 8], I16,
                                     kind="ExternalInput"),
            "slot_lo": nc.dram_tensor(f"slot_lo_{g}", [128, tl_lo], F32,
                                      kind="ExternalInput"),
            "idx_hi": nc.dram_tensor(f"idx_hi_{g}", [128, tl_hi * 8], I16,
                                     kind="ExternalInput"),
            "slot_hi": nc.dram_tensor(f"slot_hi_{g}", [128, tl_hi], F32,
                                      kind="ExternalInput"),
            "own_lo": nc.dram_tensor(f"own_lo_{g}", [128, NW * 8], I16,
                                     kind="ExternalInput"),
            "own_hi": nc.dram_tensor(f"own_hi_{g}", [128, NW * 8], I16,
                                     kind="ExternalInput"),
        }

    H = {g: nc.dram_tensor(f"H_{g}", [HROWS, ROW], F32) for g in g_names}
    xz_out = {g: nc.dram_tensor(f"xz_{g}", [128, NW * FD], F32,
                                kind="ExternalOutput") for g in g_names}
    if with_tprob:
        tp_out = nc.dram_tensor("tprobT", [2, NOWN], F32, kind="ExternalOutput")

    from contextlib import ExitStack
    with tile.TileContext(nc) as tc, ExitStack() as es:
        cpool = es.enter_context(tc.tile_pool(name="const", bufs=1))
        spool = es.enter_context(tc.tile_pool(name="streams", bufs=1))
        dq = es.enter_context(tc.tile_pool(name="densequad", bufs=3))
        dps = es.enter_context(tc.tile_pool(name="densepsum", bufs=2, space="PSUM"))
        mps = es.enter_context(tc.tile_pool(name="miscpsum", bufs=1, space="PSUM"))
        gp = es.enter_context(tc.tile_pool(name="gtiles", bufs=4))
        mp = es.enter_context(tc.tile_pool(name="m01", bufs=4))
        tp = es.enter_context(tc.tile_pool(name="tmp", bufs=4))
        wp_ = es.enter_context(tc.tile_pool(name="wpsum", bufs=2, space="PSUM"))
        pp = es.enter_context(tc.tile_pool(name="post", bufs=4))
        xp = es.enter_context(tc.tile_pool(name="xzown", bufs=1))

        W_sb = cpool.tile([kdim, FD], F32)
        nc.sync.dma_start(out=W_sb[:], in_=W_in[:])
        as_sb = cpool.tile([128, FD], F32)
        nc.sync.dma_start(out=as_sb[:], in_=as_in[:])
        ad_sb = cpool.tile([128, FD], F32)
        nc.sync.dma_start(out=ad_sb[:], in_=ad_in[:])
        b_sb = cpool.tile([128, FD], F32)
        nc.sync.dma_start(out=b_sb[:], in_=b_in[:])
        iota_sb = cpool.tile([128, 128], F32)
        nc.sync.dma_start(out=iota_sb[:], in_=iota_in[:])
        ones_row = cpool.tile([1, 128], F32)
        nc.vector.memset(ones_row[:], 1.0)
        ident = cpool.tile([128, 128], F32)
        make_identity(nc, ident[:])
        if with_tprob:
            wp_sb = cpool.tile([FD, 2], F32)
            nc.sync.dma_start(out=wp_sb[:], in_=wp_in[:])
            bp_sb = cpool.tile([128, 1], F32)
            nc.sync.dma_start(out=bp_sb[:], in_=bp_in[:])

        # zero the two pad rows of each H table
        for g in g_names:
            zrow = cpool.tile([1, ROW], F32, tag="zrow")
            nc.vector.memset(zrow[:], 0.0)
            nc.sync.dma_start(out=H[g][HALF:HALF + 1, :], in_=zrow[:])
            nc.sync.dma_start(out=H[g][HROWS - 1:HROWS, :], in_=zrow[:])

        sbufs = {}
        for g, meta in zip(g_names, metas):
            st = streams[g]
            tl_lo, tl_hi = meta["tl_lo"], meta["tl_hi"]
            ct = meta["ct"]

            idx_lo_sb = spool.tile([128, tl_lo * 8], I16, tag=f"il{g}")
            nc.sync.dma_start(out=idx_lo_sb[:], in_=st["idx_lo"][:])
            slot_lo_sb = spool.tile([128, tl_lo], F32, tag=f"sl{g}")
            nc.sync.dma_start(out=slot_lo_sb[:], in_=st["slot_lo"][:])
            idx_hi_sb = spool.tile([128, tl_hi * 8], I16, tag=f"ih{g}")
            nc.sync.dma_start(out=idx_hi_sb[:], in_=st["idx_hi"][:])
            slot_hi_sb = spool.tile([128, tl_hi], F32, tag=f"sh{g}")
            nc.sync.dma_start(out=slot_hi_sb[:], in_=st["slot_hi"][:])
            own_lo_sb = spool.tile([128, NW * 8], I16, tag=f"ol{g}")
            nc.sync.dma_start(out=own_lo_sb[:], in_=st["own_lo"][:])
            own_hi_sb = spool.tile([128, NW * 8], I16, tag=f"oh{g}")
            nc.sync.dma_start(out=own_hi_sb[:], in_=st["own_hi"][:])

            # ---------------- dense: H rows for all nodes (quads of 4 tiles)
            for q in range(NTILE // 4):
                lt = dq.tile([kdim, 512], F32, tag="lhs")
                nc.sync.dma_start(out=lt[:], in_=xT[g][:, q * 512:(q + 1) * 512])
                hp = dps.tile([128, 4, FD], F32, space="PSUM", tag="hps")
                for j in range(4):
                    nc.tensor.matmul(
                        out=hp[:, j, :], lhsT=lt[:, j * 128:(j + 1) * 128],
                        rhs=W_sb[:], start=True, stop=True)
                hq = dq.tile([128, 4, ROW], F32, tag="hq")
                nc.vector.memset(hq[:], 0.0)
                nc.vector.tensor_copy(out=hq[:, :, 0:FD], in_=hp[:])
                nc.vector.memset(hq[:, :, FD:FD + 1], 1.0)
                tmp = dq.tile([128, 4, FD], F32, tag="dtmp")
                nc.vector.tensor_tensor(
                    out=tmp[:], in0=hq[:, :, 0:FD],
                    in1=as_sb[:, None, :].to_broadcast([128, 4, FD]),
                    op=mybir.AluOpType.mult)
                nc.vector.tensor_reduce(
                    out=hq[:, :, FD + 1:FD + 2], in_=tmp[:],
                    axis=mybir.AxisListType.X, op=mybir.AluOpType.add)
                nc.vector.tensor_tensor(
                    out=tmp[:], in0=hq[:, :, 0:FD],
                    in1=ad_sb[:, None, :].to_broadcast([128, 4, FD]),
                    op=mybir.AluOpType.mult)
                nc.vector.tensor_reduce(
                    out=hq[:, :, FD + 2:FD + 3], in_=tmp[:],
                    axis=mybir.AxisListType.X, op=mybir.AluOpType.add)
                row0 = q * 512 + (1 if q * 512 >= HALF else 0)
                nc.sync.dma_start(
                    out=H[g][row0:row0 + 512, :].rearrange(
                        "(t p) r -> p t r", p=128),
                    in_=hq[:])

            sbufs[g] = (idx_lo_sb, slot_lo_sb, idx_hi_sb, slot_hi_sb,
                        own_lo_sb, own_hi_sb)

        for g, meta in zip(g_names, metas):
            ct = meta["ct"]
            (idx_lo_sb, slot_lo_sb, idx_hi_sb, slot_hi_sb,
             own_lo_sb, own_hi_sb) = sbufs[g]
            h_lo = H[g][0:HALF + 1, :]
            h_hi = H[g][HALF + 1:HROWS, :]

            # ---------------- edge phase
            xz_sb = xp.tile([128, NW * FD], F32, tag=f"xz{g}")
            lo_off = np.concatenate([[0], np.cumsum(ct[:, 0])]).astype(int)
            hi_off = np.concatenate([[0], np.cumsum(ct[:, 1])]).astype(int)
            for w in range(NW):
                # own-node rows -> a_d column for this window
                go_l = gp.tile([128, 1, ROW], F32, tag="gown")
                nc.gpsimd.dma_gather(
                    go_l[:], h_lo[:], own_lo_sb[:, w * 8:(w + 1) * 8],
                    128, 128, ROW)
                go_h = gp.tile([128, 1, ROW], F32, tag="gown")
                nc.gpsimd.dma_gather(
                    go_h[:], h_hi[:], own_hi_sb[:, w * 8:(w + 1) * 8],
                    128, 128, ROW)
                adcol = pp.tile([128, 1], F32, tag="adcol")
                nc.vector.tensor_tensor(
                    out=adcol[:], in0=go_l[:, 0, FD + 2:FD + 3],
                    in1=go_h[:, 0, FD + 2:FD + 3], op=mybir.AluOpType.add)
                # transpose [128,1] -> [1,128], then K=1 matmul to replicate
                adT = mps.tile([1, 128], F32, space="PSUM", tag="adT")
                nc.tensor.transpose(out=adT[:], in_=adcol[:], identity=ident[:])
                adT_sb = pp.tile([1, 128], F32, tag="adTs")
                nc.vector.tensor_copy(out=adT_sb[:], in_=adT[:])
                adrep_ps = mps.tile([128, 128], F32, space="PSUM", tag="adrep")
                nc.tensor.matmul(out=adrep_ps[:], lhsT=ones_row[:],
                                 rhs=adT_sb[:], start=True, stop=True)
                adrep = pp.tile([128, 128], F32, tag="adrep_sb")
                nc.vector.tensor_copy(out=adrep[:], in_=adrep_ps[:])

                psw = wp_.tile([128, FD + 1], F32, space="PSUM", tag="psw")
                n_mm = int(ct[w, 0] + ct[w, 1])
                mm_i = 0
                for h_ix, (idx_sb, slot_sb, tbl, off_a) in enumerate(
                        ((idx_lo_sb, slot_lo_sb, h_lo, lo_off),
                         (idx_hi_sb, slot_hi_sb, h_hi, hi_off))):
                    for (t0, cT) in _chunks(int(ct[w, h_ix])):
                        base = int(off_a[w]) + t0
                        g_t = gp.tile([128, GCHUNK_TILES, ROW], F32, tag="g")
                        nc.gpsimd.dma_gather(
                            g_t[:, 0:cT, :], tbl[:],
                            idx_sb[:, base * 8:(base + cT) * 8],
                            cT * 128, cT * 128, ROW)
                        m01 = mp.tile([128, GCHUNK_TILES, 128], F32, tag="m")
                        nc.vector.tensor_tensor(
                            out=m01[:, 0:cT, :],
                            in0=slot_sb[:, base:base + cT, None]
                                .to_broadcast([128, cT, 128]),
                            in1=iota_sb[:, None, :].to_broadcast([128, cT, 128]),
                            op=mybir.AluOpType.is_equal)
                        atmp = tp.tile([128, GCHUNK_TILES, 128], F32, tag="at")
                        nc.vector.tensor_tensor(
                            out=atmp[:, 0:cT, :], in0=m01[:, 0:cT, :],
                            in1=adrep[:, None, :].to_broadcast([128, cT, 128]),
                            op=mybir.AluOpType.mult)
                        ecol = tp.tile([128, GCHUNK_TILES, 1], F32, tag="ec")
                        nc.vector.tensor_reduce(
                            out=ecol[:, 0:cT, :], in_=atmp[:, 0:cT, :],
                            axis=mybir.AxisListType.X, op=mybir.AluOpType.add)
                        nc.vector.tensor_tensor(
                            out=ecol[:, 0:cT, :], in0=ecol[:, 0:cT, :],
                            in1=g_t[:, 0:cT, FD + 1:FD + 2],
                            op=mybir.AluOpType.add)
                        xcol = tp.tile([128, GCHUNK_TILES, 1], F32, tag="xc")
                        nc.vector.tensor_scalar_mul(
                            xcol[:, 0:cT, :], ecol[:, 0:cT, :], SLOPE_GAT)
                        nc.vector.tensor_tensor(
                            out=xcol[:, 0:cT, :], in0=ecol[:, 0:cT, :],
                            in1=xcol[:, 0:cT, :], op=mybir.AluOpType.max)
                        nc.scalar.activation(
                            out=xcol[:, 0:cT, :], in_=xcol[:, 0:cT, :],
                            func=mybir.ActivationFunctionType.Exp)
                        rhsw = tp.tile([128, GCHUNK_TILES, FD + 1], F32, tag="rw")
                        nc.vector.tensor_tensor(
                            out=rhsw[:, 0:cT, :], in0=g_t[:, 0:cT, 0:FD + 1],
                            in1=xcol[:, 0:cT, :].to_broadcast(
                                [128, cT, FD + 1]),
                            op=mybir.AluOpType.mult)
                        for t in range(cT):
                            nc.tensor.matmul(
                                out=psw[:], lhsT=m01[:, t, :],
                                rhs=rhsw[:, t, :],
                                start=(mm_i == 0), stop=(mm_i == n_mm - 1))
                            mm_i += 1
                # post: normalize window
                den = pp.tile([128, 1], F32, tag="den")
                nc.scalar.activation(out=den[:], in_=psw[:, FD:FD + 1],
                                     func=mybir.ActivationFunctionType.Copy,
                                     bias=1e-30)
                nc.vector.reciprocal(out=den[:], in_=den[:])
                outw = pp.tile([128, FD], F32, tag="outw")
                nc.vector.tensor_tensor(
                    out=outw[:], in0=psw[:, 0:FD],
                    in1=den[:].to_broadcast([128, FD]),
                    op=mybir.AluOpType.mult)
                nc.vector.tensor_tensor(out=outw[:], in0=outw[:], in1=b_sb[:],
                                        op=mybir.AluOpType.add)
                if relu_out:
                    nc.scalar.activation(
                        out=xz_sb[:, w * FD:(w + 1) * FD], in_=outw[:],
                        func=mybir.ActivationFunctionType.Relu)
                else:
                    nc.vector.tensor_copy(
                        out=xz_sb[:, w * FD:(w + 1) * FD], in_=outw[:])
            nc.sync.dma_start(out=xz_out[g][:], in_=xz_sb[:])

            if with_tprob and g == "r":
                for w0 in range(0, NW, 4):
                    nw_c = min(4, NW - w0)
                    cw = nw_c * 128
                    xzT = pp.tile([64, 512], F32, tag="xzT")
                    for j in range(nw_c):
                        tps_ = mps.tile([64, 128], F32, space="PSUM", tag="tT")
                        nc.tensor.transpose(
                            out=tps_[:],
                            in_=xz_sb[:, (w0 + j) * FD:(w0 + j + 1) * FD],
                            identity=ident[:])
                        nc.vector.tensor_copy(
                            out=xzT[:, j * 128:(j + 1) * 128], in_=tps_[:])
                    tpp = mps.tile([2, 512], F32, space="PSUM", tag="tpp")
                    nc.tensor.matmul(out=tpp[:, 0:cw], lhsT=wp_sb[:],
                                     rhs=xzT[:, 0:cw], start=True, stop=True)
                    tps = pp.tile([2, 512], F32, tag="tps")
                    nc.vector.tensor_tensor(
                        out=tps[:, 0:cw], in0=tpp[:2, 0:cw],
                        in1=bp_sb[:2, :].to_broadcast([2, cw]),
                        op=mybir.AluOpType.add)
                    nc.sync.dma_start(
                        out=tp_out[:, w0 * 128:w0 * 128 + cw],
                        in_=tps[:, 0:cw])

    nc.compile()
    return nc


def build_heads_launch():
    """Launch C: 4 streams of host-gathered, host-transposed rows -> MLPs.

    Per stream s: in sT [64, HEAD_PAD]; out yT [1, HEAD_PAD].
    nets: (s0->Wy1), (s1->Wy0), (s2->Wy0), (s3->Wy1)
    """
    nc = bacc.Bacc("TRN2", target_bir_lowering=False)
    ins = [nc.dram_tensor(f"s{i}", [FD, HEAD_PAD], F32, kind="ExternalInput")
           for i in range(4)]
    wys = nc.dram_tensor("WyS", [FD, FD], F32, kind="ExternalInput")
    bys = nc.dram_tensor("byS", [128, 1], F32, kind="ExternalInput")
    wnets = {}
    for nm in ("1", "0"):
        wnets[nm] = (
            nc.dram_tensor(f"Wy{nm}a", [FD, FD], F32, kind="ExternalInput"),
            nc.dram_tensor(f"by{nm}a", [128, 1], F32, kind="ExternalInput"),
            nc.dram_tensor(f"Wy{nm}b", [FD, 1], F32, kind="ExternalInput"),
            nc.dram_tensor(f"by{nm}b", [128, 1], F32, kind="ExternalInput"),
        )
    outs = [nc.dram_tensor(f"y{i}", [1, HEAD_PAD], F32, kind="ExternalOutput")
            for i in range(4)]
    net_of = ["1", "0", "0", "1"]

    with tile.TileContext(nc) as tc:
        with tc.tile_pool(name="c", bufs=1) as cp, \
             tc.tile_pool(name="s", bufs=3) as sp, \
             tc.tile_pool(name="ps", bufs=2, space="PSUM") as ps:
            wys_sb = cp.tile([FD, FD], F32)
            nc.sync.dma_start(out=wys_sb[:], in_=wys[:])
            bys_sb = cp.tile([128, 1], F32)
            nc.sync.dma_start(out=bys_sb[:], in_=bys[:])
            wsb = {}
            for nm in ("1", "0"):
                wa, ba, wb, bb = wnets[nm]
                wa_sb = cp.tile([FD, FD], F32, tag=f"wa{nm}")
                nc.sync.dma_start(out=wa_sb[:], in_=wa[:])
                ba_sb = cp.tile([128, 1], F32, tag=f"ba{nm}")
                nc.sync.dma_start(out=ba_sb[:], in_=ba[:])
                wb_sb = cp.tile([FD, 1], F32, tag=f"wb{nm}")
                nc.sync.dma_start(out=wb_sb[:], in_=wb[:])
                bb_sb = cp.tile([128, 1], F32, tag=f"bb{nm}")
                nc.sync.dma_start(out=bb_sb[:], in_=bb[:])
                wsb[nm] = (wa_sb, ba_sb, wb_sb, bb_sb)
            for i in range(4):
                wa_sb, ba_sb, wb_sb, bb_sb = wsb[net_of[i]]
                gin = sp.tile([FD, HEAD_PAD], F32, tag="gin")
                nc.sync.dma_start(out=gin[:], in_=ins[i][:])
                yrow = sp.tile([1, HEAD_PAD], F32, tag="yrow")
                for c0 in range(0, HEAD_PAD, 512):
                    cw = min(512, HEAD_PAD - c0)
                    p1 = ps.tile([FD, 512], F32, space="PSUM", tag="p1")
                    nc.tensor.matmul(out=p1[:, 0:cw], lhsT=wys_sb[:],
                                     rhs=gin[:, c0:c0 + cw],
                                     start=True, stop=True)
                    s1 = sp.tile([FD, 512], F32, tag="s1")
                    t1 = sp.tile([FD, 512], F32, tag="t1")
                    nc.vector.tensor_tensor(
                        out=s1[:, 0:cw], in0=p1[:, 0:cw],
                        in1=bys_sb[:FD, :].to_broadcast([FD, cw]),
                        op=mybir.AluOpType.add)
                    nc.vector.tensor_scalar_mul(
                        t1[:, 0:cw], s1[:, 0:cw], SLOPE_MLP)
                    nc.vector.tensor_tensor(
                        out=s1[:, 0:cw], in0=s1[:, 0:cw], in1=t1[:, 0:cw],
                        op=mybir.AluOpType.max)
                    p2 = ps.tile([FD, 512], F32, space="PSUM", tag="p2")
                    nc.tensor.matmul(out=p2[:, 0:cw], lhsT=wa_sb[:],
                                     rhs=s1[:, 0:cw], start=True, stop=True)
                    s2 = sp.tile([FD, 512], F32, tag="s2")
                    t2 = sp.tile([FD, 512], F32, tag="t2")
                    nc.vector.tensor_tensor(
                        out=s2[:, 0:cw], in0=p2[:, 0:cw],
                        in1=ba_sb[:FD, :].to_broadcast([FD, cw]),
                        op=mybir.AluOpType.add)
                    nc.vector.tensor_scalar_mul(
                        t2[:, 0:cw], s2[:, 0:cw], SLOPE_MLP)
                    nc.vector.tensor_tensor(
                        out=s2[:, 0:cw], in0=s2[:, 0:cw], in1=t2[:, 0:cw],
                        op=mybir.AluOpType.max)
                    p3 = ps.tile([1, 512], F32, space="PSUM", tag="p3")
                    nc.tensor.matmul(out=p3[:, 0:cw], lhsT=wb_sb[:],
                                     rhs=s2[:, 0:cw], start=True, stop=True)
                    s3 = sp.tile([1, 512], F32, tag="s3")
                    t3 = sp.tile([1, 512], F32, tag="t3")
                    nc.vector.tensor_tensor(
                        out=s3[:, 0:cw], in0=p3[:1, 0:cw],
                        in1=bb_sb[:1, :].to_broadcast([1, cw]),
                        op=mybir.AluOpType.add)
                    nc.vector.tensor_scalar_mul(
                        t3[:, 0:cw], s3[:, 0:cw], SLOPE_MLP)
                    nc.vector.tensor_tensor(
                        out=yrow[:, c0:c0 + cw], in0=s3[:, 0:cw],
                        in1=t3[:, 0:cw], op=mybir.AluOpType.max)
                nc.sync.dma_start(out=outs[i][:], in_=yrow[:])
    nc.compile()
    return nc


# ----------------------------------------------------------------- plumbing

def _repl_row(v, rows=128):
    v = np.asarray(v, dtype=np.float32).reshape(1, -1)
    return np.repeat(v, rows, axis=0).copy()


def _col(v):
    out = np.zeros((128, 1), dtype=np.float32)
    a = np.asarray(v, dtype=np.float32).ravel()
    out[:a.shape[0], 0] = a
    return out


def _pad_xT(x):
    """[n, d] -> padded transposed [d, NPAD] f32."""
    x = np.asarray(x, dtype=np.float32)
    out = np.zeros((x.shape[1], NPAD), dtype=np.float32)
    out[:, :x.shape[0]] = x.T
    return out


def _assemble(xz_list):
    """8x [128, NW*64] -> [NPAD, 64]."""
    full = np.zeros((NPAD, FD), dtype=np.float32)
    for k, a in enumerate(xz_list):
        blk = a.reshape(128, NW, FD).transpose(1, 0, 2).reshape(NOWN, FD)
        full[k * NOWN:(k + 1) * NOWN] = blk
    return full


def _run(nc, in_maps, label):
    res = run_bass_kernel_spmd(nc, in_maps, core_ids=list(range(NCORES)),
                               trace=_TRACE)
    if res.exec_time_ns is not None:
        LAST_EXEC_NS.append((label, res.exec_time_ns))
    return res.results


_iota = np.tile(np.arange(128, dtype=np.float32), (128, 1)).copy()


def kernel(x, edge_index, fake_x, fake_edge_index, treat_idx, control_idx,
           W1, as1, ad1, b1, W2, as2, ad2, b2,
           WyS, byS, Wy1a, by1a, Wy1b, by1b, Wy0a, by0a, Wy0b, by0b, Wp, bp):
    LAST_EXEC_NS.clear()
    meta_r, cores_r = prep_graph(edge_index)
    meta_f, cores_f = prep_graph(fake_edge_index)

    def stream_maps():
        maps = []
        for k in range(NCORES):
            m = {}
            for g, cs in (("r", cores_r), ("f", cores_f)):
                for key in ("idx_lo", "slot_lo", "idx_hi", "slot_hi",
                            "own_lo", "own_hi"):
                    m[f"{key}_{g}"] = cs[k][key]
            maps.append(m)
        return maps

    smaps = stream_maps()

    # ---- launch A: conv1
    ncA = build_conv_launch(IN_DIM, [meta_r, meta_f], relu_out=True,
                            with_tprob=False)
    common_a = {
        "xT_r": _pad_xT(x), "xT_f": _pad_xT(fake_x),
        "W": np.asarray(W1, dtype=np.float32),
        "as_r": _repl_row(as1), "ad_r": _repl_row(ad1), "b_r": _repl_row(b1),
        "iota": _iota,
    }
    in_maps = [{**common_a, **smaps[k]} for k in range(NCORES)]
    resA = _run(ncA, in_maps, "conv1")
    xz1 = _assemble([resA[k]["xz_r"] for k in range(NCORES)])
    xfz1 = _assemble([resA[k]["xz_f"] for k in range(NCORES)])

    # ---- launch B: conv2 + tprob
    ncB = build_conv_launch(FD, [meta_r, meta_f], relu_out=False,
                            with_tprob=True)
    common_b = {
        "xT_r": xz1.T.copy(), "xT_f": xfz1.T.copy(),
        "W": np.asarray(W2, dtype=np.float32),
        "as_r": _repl_row(as2), "ad_r": _repl_row(ad2), "b_r": _repl_row(b2),
        "iota": _iota,
        "Wp": np.asarray(Wp, dtype=np.float32), "bp": _col(bp),
    }
    in_maps = [{**common_b, **smaps[k]} for k in range(NCORES)]
    resB = _run(ncB, in_maps, "conv2")
    xz2 = _assemble([resB[k]["xz_r"] for k in range(NCORES)])
    xfz2 = _assemble([resB[k]["xz_f"] for k in range(NCORES)])
    tprob = np.concatenate(
        [resB[k]["tprobT"].T for k in range(NCORES)], axis=0)[:N_REAL]

    # ---- launch C: heads (host gathers + transposes rows)
    ncC = build_heads_launch()
    ti = np.asarray(treat_idx).astype(np.int64)
    ci = np.asarray(control_idx).astype(np.int64)
    gathered = [xz2[ti], xfz2[ti], xz2[ci], xfz2[ci]]
    per_core = T_CNT // NCORES  # 3125
    in_maps = []
    for k in range(NCORES):
        m = {
            "WyS": np.asarray(WyS, np.float32), "byS": _col(byS),
            "Wy1a": np.asarray(Wy1a, np.float32), "by1a": _col(by1a),
            "Wy1b": np.asarray(Wy1b, np.float32), "by1b": _col(by1b),
            "Wy0a": np.asarray(Wy0a, np.float32), "by0a": _col(by0a),
            "Wy0b": np.asarray(Wy0b, np.float32), "by0b": _col(by0b),
        }
        for i in range(4):
            sl = gathered[i][k * per_core:(k + 1) * per_core]
            buf = np.zeros((FD, HEAD_PAD), dtype=np.float32)
            buf[:, :sl.shape[0]] = sl.T
            m[f"s{i}"] = buf
        in_maps.append(m)
    resC = _run(ncC, in_maps, "heads")
    ys = []
    for i in range(4):
        ys.append(np.concatenate(
            [resC[k][f"y{i}"][0, :per_core] for k in range(NCORES)]))
    y1, yc0, y0, yc1 = ys

    return (y1, yc0, y0, yc1, xz2[:N_REAL].copy(), xfz2[:N_REAL].copy(),
            tprob)
